# revision 14
# baseline (speedup 1.0000x reference)
"""Trainium2 8-core Bass kernel for A2HNet (GIN message passing + branches).

Self-contained: computes all sharding/index structures from the inputs at
call time, builds one SPMD Bass program, runs it on cores 0-7, and gathers
the full (2048, 1) output.

Sharding: graphs (and their nodes) are block-partitioned over the 8 cores.
Each GIN layer computes y = x @ W1 locally, AllGathers y (node-major) into a
replicated table, then each core aggregates its own nodes' incoming edges
with an indirect-DMA row gather feeding one-hot matmuls that accumulate
y[dst] + sum_e y[src_e] directly in PSUM. BatchNorm batch stats are
AllGathered as per-core partial sums. The protein-conv / a2h / head branches
are computed per-core on the local 256 graphs.
"""
import numpy as np

from concourse import bacc, bass, mybir, tile
from concourse.bass_utils import run_bass_kernel_spmd
from concourse.masks import make_identity

N_CORES = 8
HID = 128
BN_EPS = 1e-5
P = 128
GCHUNK = 1  # passes per indirect gather call (128 rows)

_cache = {}


def _build_host_data(x_ligand, protein_seq, a2h, edge_index, batch_ligand):
    n_nodes = x_ligand.shape[0]
    n_graphs = a2h.shape[0]
    gpc = n_graphs // N_CORES

    batch = np.asarray(batch_ligand).astype(np.int64)
    src_all = np.asarray(edge_index)[0].astype(np.int64)
    dst_all = np.asarray(edge_index)[1].astype(np.int64)

    node_core = batch // gpc
    core_starts = np.searchsorted(node_core, np.arange(N_CORES))
    core_ends = np.searchsorted(node_core, np.arange(N_CORES), side="right")
    n_c = core_ends - core_starts
    n_pad = int(np.ceil(n_c.max() / P) * P)
    n_win = n_pad // P

    # relabel nodes per core by descending in-degree: equalizes per-window
    # edge counts across cores, shrinking max-over-cores pass padding
    deg = np.bincount(dst_all, minlength=n_nodes)
    local_old = np.arange(n_nodes) - core_starts[node_core]
    local_idx = np.empty(n_nodes, np.int64)
    perms = []
    for c in range(N_CORES):
        s_, e_ = int(core_starts[c]), int(core_ends[c])
        perm = np.argsort(-deg[s_:e_], kind="stable")
        invp = np.argsort(perm, kind="stable")
        local_idx[s_:e_] = invp
        perms.append(perm)
    gid = node_core * n_pad + local_idx

    edge_core = node_core[dst_all]
    counts = np.zeros((N_CORES, n_win), np.int64)
    per_core_edges = []
    for c in range(N_CORES):
        m = edge_core == c
        e_src = src_all[m]
        e_dst_loc = local_idx[dst_all[m]]
        order = np.argsort(e_dst_loc, kind="stable")
        e_src, e_dst_loc = e_src[order], e_dst_loc[order]
        counts[c] = np.bincount(e_dst_loc // P, minlength=n_win)
        per_core_edges.append((e_src, e_dst_loc))

    passes_w = np.maximum(1, np.ceil(counts.max(axis=0) / P).astype(np.int64))
    total_passes = int(passes_w.sum())
    rem = (-total_passes) % GCHUNK
    passes_w[-1] += rem
    total_passes += rem
    win_pass_start = np.zeros(n_win + 1, np.int64)
    win_pass_start[1:] = np.cumsum(passes_w)

    src_tiles = np.full((N_CORES, P, total_passes), 1 << 22, np.int32)
    dst_tiles = np.full((N_CORES, P, total_passes), 300.0, np.float32)
    for c in range(N_CORES):
        e_src, e_dst_loc = per_core_edges[c]
        w_of = e_dst_loc // P
        off_in_w = np.arange(len(e_src)) - np.searchsorted(w_of, w_of)
        slot = win_pass_start[w_of] * P + off_in_w
        src_tiles[c, slot % P, slot // P] = gid[e_src]
        dst_tiles[c, slot % P, slot // P] = (e_dst_loc % P).astype(np.float32)

    x_t = np.zeros((N_CORES, 128, n_pad), np.float32)
    graph_cols = np.full((N_CORES, P, n_win), 300.0, np.float32)
    xl = np.asarray(x_ligand).astype(np.float32)
    for c in range(N_CORES):
        s, e = int(core_starts[c]), int(core_ends[c])
        x_t[c, :78, : e - s] = xl[s + perms[c]].T
        col = np.full(n_pad, 300.0, np.float32)
        col[: e - s] = (batch[s + perms[c]] - c * gpc).astype(np.float32)
        graph_cols[c] = col.reshape(n_win, P).T

    prot = np.asarray(protein_seq).astype(np.float32).reshape(n_graphs, -1)
    seq_len = prot.shape[1]
    a2h_flat = np.asarray(a2h).astype(np.float32).reshape(n_graphs, -1)
    a2h_dim = a2h_flat.shape[1]
    a2h_pad = int(np.ceil(a2h_dim / P) * P)
    prot_c = np.ascontiguousarray(prot.reshape(N_CORES, gpc, seq_len))
    a2h_t = np.zeros((N_CORES, a2h_pad, gpc), np.float32)
    for c in range(N_CORES):
        a2h_t[c, :a2h_dim, :] = a2h_flat[c * gpc : (c + 1) * gpc].T

    x_gath = np.zeros((N_CORES, 78, total_passes * P), np.float32)
    inv = np.full(N_CORES * n_pad, -1, np.int64)
    for c in range(N_CORES):
        s_, e_ = int(core_starts[c]), int(core_ends[c])
        inv[c * n_pad : c * n_pad + (e_ - s_)] = s_ + perms[c]
    for c in range(N_CORES):
        st = src_tiles[c]  # (P, total_passes), gid or 1<<22
        flat = st.T.reshape(-1)  # slot (pi, p) at pi*P + p
        valid = flat < N_CORES * n_pad
        rows = inv[flat[valid]]
        cols = np.where(valid)[0]
        x_gath[c][:, cols] = xl[rows].T
    meta = dict(
        n_pad=n_pad, n_win=n_win, gpc=gpc, n_nodes=n_nodes,
        total_passes=total_passes, win_pass_start=win_pass_start,
        seq_len=seq_len, a2h_pad=a2h_pad, n_c=n_c, a2h_dim=a2h_dim,
    )
    data = dict(
        x_t=x_t, src_tiles=src_tiles, dst_tiles=dst_tiles,
        graph_cols=graph_cols, prot_c=prot_c, a2h_t=a2h_t, x_gath=x_gath,
    )
    return meta, data


def _build_program(meta):
    n_pad, n_win, gpc = meta["n_pad"], meta["n_win"], meta["gpc"]
    total_passes = meta["total_passes"]
    wps = meta["win_pass_start"]
    seq_len, a2h_pad = meta["seq_len"], meta["a2h_pad"]
    n_nodes = meta["n_nodes"]
    conv_w = seq_len - 8 + 1
    g_grp = gpc // 4
    f32 = mybir.dt.float32

    nc = bacc.Bacc(None, target_bir_lowering=False, num_devices=N_CORES)

    def par(name, shape, dt=f32):
        return nc.declare_dram_parameter(name, list(shape), dt, isOutput=False)

    x_t_d = par("x_t", (128, n_pad))
    src_d = par("src_tiles", (P, total_passes), mybir.dt.int32)
    dst_d = par("dst_tiles", (P, total_passes), mybir.dt.bfloat16)
    gcol_d = par("graph_cols", (P, n_win))
    prot_d = par("prot_c", (gpc, seq_len))
    a2h_d = par("a2h_t", (a2h_pad, gpc))
    iota_d = par("iota128", (P, P), mybir.dt.bfloat16)
    iota256_d = par("iota256", (P, 256))
    kblk_d = par("kblk", (32, 128))
    convb_d = par("convb_col", (128, 1))
    nreal_d = par("n_real_col", (P, 1))
    xgath_d = par("x_gath", (78, total_passes * P))
    w_d = {}
    for i in range(3):
        din = 78 if i == 0 else HID
        for nm, shp in [
            (f"g{i}W1", (din, HID)), (f"g{i}b1", (HID, 1)),
            (f"g{i}W2", (HID, HID)), (f"g{i}b2", (HID, 1)),
            (f"bn{i}g", (HID, 1)), (f"bn{i}b", (HID, 1)),
        ]:
            w_d[nm] = par(nm, shp)
    for nm, shp in [
        ("ligW", (HID, HID)), ("ligb", (HID, 1)),
        ("protW", (32, HID)), ("protb", (HID, 1)),
        ("a2h1W", (a2h_pad, HID)), ("a2h1b", (HID, 1)),
        ("a2h2W", (HID, HID)), ("a2h2b", (HID, 1)),
        ("c1W00", (HID, HID)), ("c1W10", (HID, HID)), ("c1W20", (HID, HID)),
        ("c1W01", (HID, HID)), ("c1W11", (HID, HID)), ("c1W21", (HID, HID)),
        ("c1ba", (HID, 1)), ("c1bb", (HID, 1)),
        ("c2Wa", (HID, HID)), ("c2Wb", (HID, HID)), ("c2b", (HID, 1)),
        ("outW", (HID, 1)), ("outb", (1, 1)),
    ]:
        w_d[nm] = par(nm, shp)
    out_d = nc.declare_dram_parameter("out", [gpc, 1], f32, isOutput=True)

    bf16 = mybir.dt.bfloat16
    y_nm = nc.dram_tensor("y_nm", [n_pad, HID], bf16)
    y_full = nc.dram_tensor(
        "y_full", [N_CORES * n_pad, HID], bf16, addr_space="Shared"
    )
    stats_in = nc.dram_tensor("stats_in", [P, 2], f32)
    stats_out = nc.dram_tensor(
        "stats_out", [N_CORES * P, 2], f32, addr_space="Shared"
    )
    rg = [list(range(N_CORES))]

    with tile.TileContext(nc) as tc:
        with (
            tc.tile_pool(name="persist", bufs=1) as pp,
            tc.tile_pool(name="work", bufs=2) as wp,
            tc.tile_pool(name="patchp", bufs=1) as patchp,
            tc.tile_pool(name="gpool", bufs=12) as gp,
            tc.tile_pool(name="spool", bufs=8) as sp,
            tc.tile_pool(name="ypool", bufs=3) as yp,
            tc.tile_pool(name="mm128", bufs=3, space="PSUM") as mmp,
            tc.tile_pool(name="h2p", bufs=2, space="PSUM") as h2p,
            tc.tile_pool(name="convp", bufs=1, space="PSUM") as convp,
            tc.tile_pool(name="brp", bufs=1, space="PSUM") as brp,
            tc.tile_pool(name="xgp", bufs=1, space="PSUM") as xgp,
        ):
            # ---- persistent loads ----
            cur = pp.tile([128, n_pad], f32, tag="cur")
            nc.sync.dma_start(cur[:], x_t_d.ap())
            h1_t = pp.tile([128, n_pad], f32)
            src_t = pp.tile([P, total_passes], mybir.dt.int32)
            nc.sync.dma_start(src_t[:], src_d.ap())
            dst_t = pp.tile([P, total_passes], mybir.dt.bfloat16)
            nc.sync.dma_start(dst_t[:], dst_d.ap())
            gcol_t = pp.tile([P, n_win], f32)
            nc.sync.dma_start(gcol_t[:], gcol_d.ap())
            iota_t = pp.tile([P, P], mybir.dt.bfloat16)
            nc.sync.dma_start(iota_t[:], iota_d.ap())
            iota256_t = pp.tile([P, 256], f32)
            nc.sync.dma_start(iota256_t[:], iota256_d.ap())
            kblk_t = pp.tile([32, 128], f32)
            nc.sync.dma_start(kblk_t[:], kblk_d.ap())
            convb_t = pp.tile([128, 1], f32)
            nc.sync.dma_start(convb_t[:], convb_d.ap())
            nrc_t = pp.tile([P, 1], f32)
            nc.sync.dma_start(nrc_t[:], nreal_d.ap())
            for _gi in range(12):
                gz = gp.tile([P, HID], mybir.dt.bfloat16, tag="gt", name=f"gz{_gi}")
                nc.vector.memset(gz[:], 0.0)
            wt = {}
            for k, d in w_d.items():
                if k == "a2h1W":
                    continue
                wt[k] = pp.tile(list(d.shape), f32, tag=f"w_{k}", name=f"w_{k}")
                nc.sync.dma_start(wt[k][:], d.ap())
            ident = pp.tile([P, P], f32)
            make_identity(nc, ident[:])

            # ================= protein conv branch =================
            maxes = pp.tile([128, g_grp], f32)
            for grp in range(g_grp):
                patch = patchp.tile([32, conv_w], f32, tag="patch")
                sl = prot_d.ap()[4 * grp : 4 * grp + 4, :]
                src_ap = bass.AP(
                    sl.tensor, sl.offset, [list(sl.ap[0]), [1, 8], [1, conv_w]]
                )
                nc.sync.dma_start(patch[:], src_ap)
                m2 = wp.tile([128, 2], f32, tag="m2")
                for half, (c0, c1) in enumerate([(0, 512), (512, conv_w)]):
                    cps = convp.tile([128, 512], f32, tag="convps")
                    nc.tensor.matmul(
                        cps[:, : c1 - c0], kblk_t[:], patch[:, c0:c1],
                        start=True, stop=True,
                    )
                    nc.vector.tensor_reduce(
                        m2[:, half : half + 1], cps[:, : c1 - c0],
                        mybir.AxisListType.X, mybir.AluOpType.max,
                    )
                nc.vector.tensor_reduce(
                    maxes[:, grp : grp + 1], m2[:], mybir.AxisListType.X,
                    mybir.AluOpType.max,
                )
            xp_m = pp.tile([128, g_grp], f32)
            nc.scalar.activation(
                xp_m[:], maxes[:], mybir.ActivationFunctionType.Relu,
                bias=convb_t[:], scale=1.0,
            )
            xp_ch = pp.tile([32, gpc], f32)
            xp_ch_v = xp_ch[:].rearrange("c (g gi) -> c g gi", gi=4)
            for gi in range(4):
                nc.sync.dma_start(
                    xp_ch_v[:, :, gi], xp_m[gi * 32 : (gi + 1) * 32, :]
                )
            xp_t = pp.tile([128, gpc], f32)
            pps = brp.tile([128, gpc], f32, tag="brps")
            nc.tensor.matmul(pps[:], wt["protW"][:], xp_ch[:], start=True, stop=True)
            nc.scalar.activation(
                xp_t[:], pps[:], mybir.ActivationFunctionType.Relu,
                bias=wt["protb"][:], scale=1.0,
            )

            # ================= a2h branch =================
            xa1 = pp.tile([128, gpc], f32)
            aps = brp.tile([128, gpc], f32, tag="brps")
            n_chunk = a2h_pad // P
            for ch in range(n_chunk):
                a_tile = wp.tile([P, gpc], f32, tag="a2h")
                nc.sync.dma_start(a_tile[:], a2h_d.ap()[ch * P : (ch + 1) * P, :])
                aw = wp.tile([P, HID], f32, tag="a2hw")
                nc.sync.dma_start(
                    aw[:], w_d["a2h1W"].ap()[ch * P : (ch + 1) * P, :]
                )
                nc.tensor.matmul(
                    aps[:], aw[:], a_tile[:],
                    start=(ch == 0), stop=(ch == n_chunk - 1),
                )
            nc.scalar.activation(
                xa1[:], aps[:], mybir.ActivationFunctionType.Relu,
                bias=wt["a2h1b"][:], scale=1.0,
            )
            xa_t = pp.tile([128, gpc], f32)
            aps2 = brp.tile([128, gpc], f32, tag="brps")
            nc.tensor.matmul(aps2[:], wt["a2h2W"][:], xa1[:], start=True, stop=True)
            nc.scalar.activation(
                xa_t[:], aps2[:], mybir.ActivationFunctionType.Relu,
                bias=wt["a2h2b"][:], scale=1.0,
            )

            # ================= GIN layers =================
            # xpad_col: value of pad-node activation columns (constant per
            # feature). Starts at zero (host zero-pads x).
            xpad = pp.tile([128, 1], f32)
            nc.vector.memset(xpad[:], 0.0)
            for li in range(3):
                kdim = 78 if li == 0 else HID
                W1, W2 = wt[f"g{li}W1"], wt[f"g{li}W2"]
                b1, b2 = wt[f"g{li}b1"], wt[f"g{li}b2"]
                # ---- y node-major -> DRAM, then AllGather (layers 1,2) ----
                if li > 0:
                    for w in range(n_win):
                        yps = mmp.tile([P, HID], f32, tag="mm128")
                        nc.tensor.matmul(
                            yps[:], cur[:kdim, w * P : (w + 1) * P],
                            W1[:kdim, :], start=True, stop=True,
                        )
                        y_sb = yp.tile([P, HID], mybir.dt.bfloat16, tag="ysb")
                        nc.vector.tensor_copy(y_sb[:], yps[:])
                        nc.sync.dma_start(y_nm[w * P : (w + 1) * P, :], y_sb[:])
                    nc.gpsimd.collective_compute(
                        "AllGather", mybir.AluOpType.bypass, replica_groups=rg,
                        ins=[y_nm.ap().opt()], outs=[y_full.ap().opt()],
                    )
                # ---- aggregation ----
                g_tiles = {}
                xg_blks = {}
                for w in range(n_win):
                    agg = mmp.tile([128, P], f32, tag="mm128")
                    nc.tensor.matmul(
                        agg[:], W1[:kdim, :], cur[:kdim, w * P : (w + 1) * P],
                        start=True, stop=False,
                    )
                    p0, p1 = int(wps[w]), int(wps[w + 1])
                    for pi in range(p0, p1):
                        ck = pi // GCHUNK
                        if ck not in g_tiles:
                            gt = gp.tile([P, HID], mybir.dt.bfloat16, tag="gt")
                            if li == 0:
                                blk8 = ck // 8
                                if blk8 not in xg_blks:
                                    xg_sl = wp.tile(
                                        [78, 8 * P], f32, tag="xgsl"
                                    )
                                    lo = blk8 * 8 * P
                                    hi = min(lo + 8 * P, total_passes * P)
                                    nc.sync.dma_start(
                                        xg_sl[:, : hi - lo],
                                        xgath_d.ap()[:, lo:hi],
                                    )
                                    xg_blks = {blk8: xg_sl}
                                xs = xg_blks[blk8]
                                o = (ck % 8) * P
                                gps_t = h2p.tile([128, 512], f32, tag="h2ps")
                                nc.tensor.matmul(
                                    gps_t[:, :HID], xs[:, o : o + P],
                                    W1[:kdim, :], start=True, stop=True,
                                )
                                nc.vector.tensor_copy(gt[:], gps_t[:, :HID])
                            else:
                                nc.gpsimd.indirect_dma_start(
                                    out=gt[:],
                                    out_offset=None,
                                    in_=y_full.ap(),
                                    in_offset=bass.IndirectOffsetOnAxis(
                                        ap=src_t[:, ck : ck + 1],
                                        axis=0,
                                    ),
                                    bounds_check=N_CORES * n_pad - 1,
                                    oob_is_err=False,
                                )
                            g_tiles = {ck: gt}
                        s_tile = sp.tile([P, P], mybir.dt.bfloat16, tag="s1h")
                        nc.vector.tensor_tensor(
                            out=s_tile[:],
                            in0=iota_t[:],
                            in1=dst_t[:, pi : pi + 1].to_broadcast([P, P]),
                            op=mybir.AluOpType.is_equal,
                        )
                        nc.tensor.matmul(
                            agg[:], g_tiles[ck][:], s_tile[:],
                            start=False, stop=(pi == p1 - 1),
                        )
                    nc.scalar.activation(
                        h1_t[:, w * P : (w + 1) * P], agg[:],
                        mybir.ActivationFunctionType.Relu,
                        bias=b1[:], scale=1.0,
                    )
                # pad-column constants: ypad = W1^T xpad ; h1pad = relu(+b1)
                ypps = mmp.tile([128, 1], f32, tag="mm128")
                nc.tensor.matmul(
                    ypps[:, 0:1], W1[:kdim, :], xpad[:kdim, 0:1],
                    start=True, stop=True,
                )
                h1pad = wp.tile([128, 1], f32, tag="h1pad")
                nc.scalar.activation(
                    h1pad[:], ypps[:, 0:1],
                    mybir.ActivationFunctionType.Relu, bias=b1[:], scale=1.0,
                )
                vpps = mmp.tile([128, 1], f32, tag="mm128")
                nc.tensor.matmul(
                    vpps[:, 0:1], W2[:], h1pad[:], start=True, stop=True
                )
                vpad = wp.tile([128, 1], f32, tag="vpad")
                nc.vector.tensor_copy(vpad[:], vpps[:, 0:1])
                # ---- W2 pass 1: partial sums / sumsq ----
                n_big = (n_pad + 511) // 512
                parts = wp.tile([128, 2 * n_big], f32, tag="parts")
                for b in range(n_big):
                    c0, c1 = b * 512, min((b + 1) * 512, n_pad)
                    wl = c1 - c0
                    h2ps = h2p.tile([128, 512], f32, tag="h2ps")
                    nc.tensor.matmul(
                        h2ps[:, :wl], W2[:], h1_t[:, c0:c1],
                        start=True, stop=True,
                    )
                    sq = wp.tile([128, 512], f32, tag="sq")
                    nc.scalar.activation(
                        sq[:, :wl], h2ps[:, :wl],
                        mybir.ActivationFunctionType.Square,
                    )
                    nc.vector.tensor_reduce(
                        parts[:, 2 * b : 2 * b + 1], h2ps[:, :wl],
                        mybir.AxisListType.X, mybir.AluOpType.add,
                    )
                    nc.vector.tensor_reduce(
                        parts[:, 2 * b + 1 : 2 * b + 2], sq[:, :wl],
                        mybir.AxisListType.X, mybir.AluOpType.add,
                    )
                stats = wp.tile([128, 2], f32, tag="stats")
                pv = parts[:].rearrange("p (b s) -> p s b", s=2)
                nc.vector.tensor_reduce(
                    stats[:, 0:1], pv[:, 0, :], mybir.AxisListType.X,
                    mybir.AluOpType.add,
                )
                nc.vector.tensor_reduce(
                    stats[:, 1:2], pv[:, 1, :], mybir.AxisListType.X,
                    mybir.AluOpType.add,
                )
                # subtract pad-column contribution: ndead * [v, v^2]
                ndead = wp.tile([128, 1], f32, tag="ndead")
                nc.vector.tensor_scalar(
                    out=ndead[:], in0=nrc_t[:], scalar1=-1.0,
                    scalar2=float(n_pad), op0=mybir.AluOpType.mult,
                    op1=mybir.AluOpType.add,
                )
                vsq = wp.tile([128, 1], f32, tag="vsq")
                nc.vector.tensor_tensor(
                    out=vsq[:], in0=vpad[:], in1=vpad[:],
                    op=mybir.AluOpType.mult,
                )
                corr = wp.tile([128, 2], f32, tag="corr")
                nc.vector.tensor_tensor(
                    out=corr[:, 0:1], in0=vpad[:], in1=ndead[:],
                    op=mybir.AluOpType.mult,
                )
                nc.vector.tensor_tensor(
                    out=corr[:, 1:2], in0=vsq[:], in1=ndead[:],
                    op=mybir.AluOpType.mult,
                )
                nc.vector.tensor_tensor(
                    out=stats[:], in0=stats[:], in1=corr[:],
                    op=mybir.AluOpType.subtract,
                )
                nc.sync.dma_start(stats_in[:, :], stats[:])
                nc.gpsimd.collective_compute(
                    "AllGather", mybir.AluOpType.bypass, replica_groups=rg,
                    ins=[stats_in.ap().opt()], outs=[stats_out.ap().opt()],
                )
                allst = wp.tile([128, N_CORES, 2], f32, tag="allst")
                nc.sync.dma_start(
                    allst[:], stats_out.ap().rearrange("(c p) s -> p c s", p=P)
                )
                tot = wp.tile([128, 2], f32, tag="tot")
                av = allst[:].rearrange("p c s -> p s c")
                nc.vector.tensor_reduce(
                    tot[:, 0:1], av[:, 0, :], mybir.AxisListType.X,
                    mybir.AluOpType.add,
                )
                nc.vector.tensor_reduce(
                    tot[:, 1:2], av[:, 1, :], mybir.AxisListType.X,
                    mybir.AluOpType.add,
                )
                inv_n = 1.0 / float(n_nodes)
                mu = wp.tile([128, 1], f32, tag="mu")
                nc.vector.tensor_scalar(
                    out=mu[:], in0=tot[:, 0:1], scalar1=inv_n, scalar2=None,
                    op0=mybir.AluOpType.mult,
                )
                var = wp.tile([128, 1], f32, tag="var")
                nc.vector.tensor_scalar(
                    out=var[:], in0=tot[:, 1:2], scalar1=inv_n, scalar2=None,
                    op0=mybir.AluOpType.mult,
                )
                musq = wp.tile([128, 1], f32, tag="musq")
                nc.vector.tensor_tensor(
                    out=musq[:], in0=mu[:], in1=mu[:], op=mybir.AluOpType.mult
                )
                nc.vector.tensor_tensor(
                    out=var[:], in0=var[:], in1=musq[:],
                    op=mybir.AluOpType.subtract,
                )
                nc.vector.tensor_scalar(
                    out=var[:], in0=var[:], scalar1=BN_EPS, scalar2=None,
                    op0=mybir.AluOpType.add,
                )
                sd = wp.tile([128, 1], f32, tag="sd")
                nc.scalar.sqrt(sd[:], var[:])
                inv_sd = wp.tile([128, 1], f32, tag="invsd")
                nc.vector.reciprocal(inv_sd[:], sd[:])
                A = wp.tile([128, 1], f32, tag="A")
                nc.vector.tensor_tensor(
                    out=A[:], in0=inv_sd[:], in1=wt[f"bn{li}g"][:],
                    op=mybir.AluOpType.mult,
                )
                negmuA = wp.tile([128, 1], f32, tag="negmuA")
                nc.vector.tensor_tensor(
                    out=negmuA[:], in0=mu[:], in1=A[:], op=mybir.AluOpType.mult
                )
                B = wp.tile([128, 1], f32, tag="B")
                nc.vector.tensor_tensor(
                    out=B[:], in0=wt[f"bn{li}b"][:], in1=negmuA[:],
                    op=mybir.AluOpType.subtract,
                )
                # ---- W2 pass 2 + BN + relu (in place: cur is dead) ----
                nxt = cur
                for b in range(n_big):
                    c0, c1 = b * 512, min((b + 1) * 512, n_pad)
                    wl = c1 - c0
                    h2ps = h2p.tile([128, 512], f32, tag="h2ps")
                    nc.tensor.matmul(
                        h2ps[:, :wl], W2[:], h1_t[:, c0:c1],
                        start=True, stop=True,
                    )
                    nc.scalar.activation(
                        nxt[:, c0:c1], h2ps[:, :wl],
                        mybir.ActivationFunctionType.Relu,
                        bias=B[:], scale=A[:],
                    )
                # update pad-column constant for next layer (in place)
                nc.scalar.activation(
                    xpad[:], vpad[:], mybir.ActivationFunctionType.Relu,
                    bias=B[:], scale=A[:],
                )
                cur = nxt

            # ================= pooling + lig MLP =================
            xg_ps = xgp.tile([128, gpc], f32, tag="xgps")
            for w in range(n_win):
                tps = mmp.tile([P, P], f32, tag="mm128")
                nc.tensor.transpose(tps[:], cur[:, w * P : (w + 1) * P], ident[:])
                x_nm = wp.tile([P, P], f32, tag="xnm")
                nc.vector.tensor_copy(x_nm[:], tps[:])
                pool1h = sp.tile([P, 256], f32, tag="pool1h")
                nc.vector.tensor_tensor(
                    out=pool1h[:],
                    in0=iota256_t[:],
                    in1=gcol_t[:, w : w + 1].to_broadcast([P, 256]),
                    op=mybir.AluOpType.is_equal,
                )
                nc.tensor.matmul(
                    xg_ps[:], x_nm[:], pool1h[:, :gpc],
                    start=(w == 0), stop=(w == n_win - 1),
                )
            xg_sb = pp.tile([128, gpc], f32)
            nc.vector.tensor_copy(xg_sb[:], xg_ps[:])
            lps = brp.tile([128, gpc], f32, tag="brps")
            nc.tensor.matmul(lps[:], wt["ligW"][:], xg_sb[:], start=True, stop=True)
            xg_t = pp.tile([128, gpc], f32)
            nc.scalar.activation(
                xg_t[:], lps[:], mybir.ActivationFunctionType.Relu,
                bias=wt["ligb"][:], scale=1.0,
            )

            # ================= head =================
            branches = [xg_t, xp_t, xa_t]
            xc1a = pp.tile([128, gpc], f32)
            xc1b = pp.tile([128, gpc], f32)
            for h, (xc1h, bkey) in enumerate(
                [(xc1a, "c1ba"), (xc1b, "c1bb")]
            ):
                hps = brp.tile([128, gpc], f32, tag="brps")
                for blk in range(3):
                    nc.tensor.matmul(
                        hps[:], wt[f"c1W{blk}{h}"][:],
                        branches[blk][:],
                        start=(blk == 0), stop=(blk == 2),
                    )
                nc.scalar.activation(
                    xc1h[:], hps[:], mybir.ActivationFunctionType.Relu,
                    bias=wt[bkey][:], scale=1.0,
                )
            c2ps = brp.tile([128, gpc], f32, tag="brps")
            nc.tensor.matmul(c2ps[:], wt["c2Wa"][:], xc1a[:], start=True, stop=False)
            nc.tensor.matmul(c2ps[:], wt["c2Wb"][:], xc1b[:], start=False, stop=True)
            xc2 = pp.tile([128, gpc], f32)
            nc.scalar.activation(
                xc2[:], c2ps[:], mybir.ActivationFunctionType.Relu,
                bias=wt["c2b"][:], scale=1.0,
            )
            ops = brp.tile([128, gpc], f32, tag="brps")
            nc.tensor.matmul(ops[:1, :], wt["outW"][:], xc2[:], start=True, stop=True)
            ovec = wp.tile([1, gpc], f32, tag="ovec")
            nc.scalar.activation(
                ovec[:], ops[:1, :], mybir.ActivationFunctionType.Identity,
                bias=wt["outb"][:], scale=1.0,
            )
            nc.sync.dma_start(out_d.ap(), ovec[:])

    return nc


def kernel(**inputs):
    meta, data = _build_host_data(
        inputs["x_ligand"], inputs["protein_seq"], inputs["a2h"],
        inputs["edge_index"], inputs["batch_ligand"],
    )
    gpc = meta["gpc"]
    key = (meta["n_pad"], meta["total_passes"], gpc, meta["seq_len"])
    if key not in _cache:
        nc = _build_program(meta)
        nc.compile()
        _cache[key] = nc
    nc = _cache[key]

    f32 = np.float32
    bf = np.dtype("bfloat16")
    iota128 = np.tile(np.arange(P, dtype=f32), (P, 1)).astype(bf)
    iota256 = np.tile(np.arange(256, dtype=f32), (P, 1))
    conv_k = np.asarray(inputs["conv_k"], f32)
    kblk = np.zeros((32, 128), f32)
    for g in range(4):
        kblk[g * 8 : (g + 1) * 8, g * 32 : (g + 1) * 32] = conv_k[:, 0, :].T
    convb_col = np.tile(np.asarray(inputs["conv_b"], f32), 4).reshape(128, 1)

    shared = {
        "iota128": iota128, "iota256": iota256, "kblk": kblk,
        "convb_col": convb_col,
    }
    col = lambda v: np.asarray(v, f32).reshape(-1, 1)
    for i in range(3):
        shared[f"g{i}W1"] = np.asarray(inputs[f"gin{i}_W1"], f32)
        shared[f"g{i}b1"] = col(inputs[f"gin{i}_b1"])
        shared[f"g{i}W2"] = np.asarray(inputs[f"gin{i}_W2"], f32)
        shared[f"g{i}b2"] = col(inputs[f"gin{i}_b2"])
        shared[f"bn{i}g"] = col(inputs[f"bn{i}_g"])
        shared[f"bn{i}b"] = col(inputs[f"bn{i}_b"])
    shared["ligW"] = np.asarray(inputs["lig_W"], f32)
    shared["ligb"] = col(inputs["lig_b"])
    shared["protW"] = np.asarray(inputs["prot_W"], f32)
    shared["protb"] = col(inputs["prot_b"])
    a2h1W = np.zeros((meta["a2h_pad"], HID), f32)
    a2h1W[: meta["a2h_dim"]] = np.asarray(inputs["a2h1_W"], f32)
    shared["a2h1W"] = a2h1W
    shared["a2h1b"] = col(inputs["a2h1_b"])
    shared["a2h2W"] = np.asarray(inputs["a2h2_W"], f32)
    shared["a2h2b"] = col(inputs["a2h2_b"])
    c1W = np.asarray(inputs["c1_W"], f32)
    for blk in range(3):
        for h in range(2):
            shared[f"c1W{blk}{h}"] = np.ascontiguousarray(
                c1W[blk * HID : (blk + 1) * HID, h * HID : (h + 1) * HID]
            )
    c1b = np.asarray(inputs["c1_b"], f32)
    shared["c1ba"] = col(c1b[:HID])
    shared["c1bb"] = col(c1b[HID:])
    c2W = np.asarray(inputs["c2_W"], f32)
    shared["c2Wa"] = np.ascontiguousarray(c2W[:HID, :])
    shared["c2Wb"] = np.ascontiguousarray(c2W[HID:, :])
    shared["c2b"] = col(inputs["c2_b"])
    shared["outW"] = np.asarray(inputs["out_W"], f32)
    shared["outb"] = np.asarray(inputs["out_b"], f32).reshape(1, 1)

    in_maps = []
    for c in range(N_CORES):
        m = dict(shared)
        m["x_t"] = data["x_t"][c]
        m["src_tiles"] = data["src_tiles"][c]
        m["dst_tiles"] = data["dst_tiles"][c].astype(np.dtype("bfloat16"))
        m["graph_cols"] = data["graph_cols"][c]
        m["prot_c"] = data["prot_c"][c]
        m["a2h_t"] = data["a2h_t"][c]
        m["n_real_col"] = np.full((P, 1), float(meta["n_c"][c]), f32)
        m["x_gath"] = data["x_gath"][c]
        in_maps.append(m)

    res = run_bass_kernel_spmd(nc, in_maps, core_ids=list(range(N_CORES)))
    outs = [
        np.asarray(res.results[c]["out"]).reshape(gpc, 1)
        for c in range(N_CORES)
    ]
    return np.concatenate(outs, axis=0).astype(np.float32)


# revision 15
# speedup vs baseline: 1.0318x; 1.0318x over previous
"""Trainium2 8-core Bass kernel for A2HNet (GIN message passing + branches).

Self-contained: computes all sharding/index structures from the inputs at
call time, builds one SPMD Bass program, runs it on cores 0-7, and gathers
the full (2048, 1) output.

Sharding: graphs (and their nodes) are block-partitioned over the 8 cores.
Each GIN layer computes y = x @ W1 locally, AllGathers y (node-major) into a
replicated table, then each core aggregates its own nodes' incoming edges
with an indirect-DMA row gather feeding one-hot matmuls that accumulate
y[dst] + sum_e y[src_e] directly in PSUM. BatchNorm batch stats are
AllGathered as per-core partial sums. The protein-conv / a2h / head branches
are computed per-core on the local 256 graphs.
"""
import numpy as np

from concourse import bacc, bass, mybir, tile
from concourse.bass_utils import run_bass_kernel_spmd
from concourse.masks import make_identity

N_CORES = 8
HID = 128
BN_EPS = 1e-5
P = 128
GCHUNK = 1  # passes per indirect gather call (128 rows)

_cache = {}


def _build_host_data(x_ligand, protein_seq, a2h, edge_index, batch_ligand):
    n_nodes = x_ligand.shape[0]
    n_graphs = a2h.shape[0]
    gpc = n_graphs // N_CORES

    batch = np.asarray(batch_ligand).astype(np.int64)
    src_all = np.asarray(edge_index)[0].astype(np.int64)
    dst_all = np.asarray(edge_index)[1].astype(np.int64)

    node_core = batch // gpc
    core_starts = np.searchsorted(node_core, np.arange(N_CORES))
    core_ends = np.searchsorted(node_core, np.arange(N_CORES), side="right")
    n_c = core_ends - core_starts
    n_pad = int(np.ceil(n_c.max() / P) * P)
    n_win = n_pad // P

    # relabel nodes per core by descending in-degree: equalizes per-window
    # edge counts across cores, shrinking max-over-cores pass padding
    deg = np.bincount(dst_all, minlength=n_nodes)
    local_old = np.arange(n_nodes) - core_starts[node_core]
    local_idx = np.empty(n_nodes, np.int64)
    perms = []
    for c in range(N_CORES):
        s_, e_ = int(core_starts[c]), int(core_ends[c])
        perm = np.argsort(-deg[s_:e_], kind="stable")
        invp = np.argsort(perm, kind="stable")
        local_idx[s_:e_] = invp
        perms.append(perm)
    gid = node_core * n_pad + local_idx

    edge_core = node_core[dst_all]
    counts = np.zeros((N_CORES, n_win), np.int64)
    per_core_edges = []
    for c in range(N_CORES):
        m = edge_core == c
        e_src = src_all[m]
        e_dst_loc = local_idx[dst_all[m]]
        order = np.argsort(e_dst_loc, kind="stable")
        e_src, e_dst_loc = e_src[order], e_dst_loc[order]
        counts[c] = np.bincount(e_dst_loc // P, minlength=n_win)
        per_core_edges.append((e_src, e_dst_loc))

    passes_w = np.maximum(1, np.ceil(counts.max(axis=0) / P).astype(np.int64))
    total_passes = int(passes_w.sum())
    rem = (-total_passes) % GCHUNK
    passes_w[-1] += rem
    total_passes += rem
    win_pass_start = np.zeros(n_win + 1, np.int64)
    win_pass_start[1:] = np.cumsum(passes_w)

    src_tiles = np.full((N_CORES, P, total_passes), 1 << 22, np.int32)
    dst_tiles = np.full((N_CORES, P, total_passes), 300.0, np.float32)
    for c in range(N_CORES):
        e_src, e_dst_loc = per_core_edges[c]
        w_of = e_dst_loc // P
        off_in_w = np.arange(len(e_src)) - np.searchsorted(w_of, w_of)
        slot = win_pass_start[w_of] * P + off_in_w
        src_tiles[c, slot % P, slot // P] = gid[e_src]
        dst_tiles[c, slot % P, slot // P] = (e_dst_loc % P).astype(np.float32)

    x_t = np.zeros((N_CORES, 128, n_pad), np.float32)
    graph_cols = np.full((N_CORES, P, n_win), 300.0, np.float32)
    xl = np.asarray(x_ligand).astype(np.float32)
    for c in range(N_CORES):
        s, e = int(core_starts[c]), int(core_ends[c])
        x_t[c, :78, : e - s] = xl[s + perms[c]].T
        col = np.full(n_pad, 300.0, np.float32)
        col[: e - s] = (batch[s + perms[c]] - c * gpc).astype(np.float32)
        graph_cols[c] = col.reshape(n_win, P).T

    prot = np.asarray(protein_seq).astype(np.float32).reshape(n_graphs, -1)
    seq_len = prot.shape[1]
    a2h_flat = np.asarray(a2h).astype(np.float32).reshape(n_graphs, -1)
    a2h_dim = a2h_flat.shape[1]
    a2h_pad = int(np.ceil(a2h_dim / P) * P)
    prot_c = np.ascontiguousarray(prot.reshape(N_CORES, gpc, seq_len))
    a2h_t = np.zeros((N_CORES, a2h_pad, gpc), np.float32)
    for c in range(N_CORES):
        a2h_t[c, :a2h_dim, :] = a2h_flat[c * gpc : (c + 1) * gpc].T

    x_gath = np.zeros((N_CORES, 78, total_passes * P), np.float32)
    inv = np.full(N_CORES * n_pad, -1, np.int64)
    for c in range(N_CORES):
        s_, e_ = int(core_starts[c]), int(core_ends[c])
        inv[c * n_pad : c * n_pad + (e_ - s_)] = s_ + perms[c]
    for c in range(N_CORES):
        st = src_tiles[c]  # (P, total_passes), gid or 1<<22
        flat = st.T.reshape(-1)  # slot (pi, p) at pi*P + p
        valid = flat < N_CORES * n_pad
        rows = inv[flat[valid]]
        cols = np.where(valid)[0]
        x_gath[c][:, cols] = xl[rows].T
    meta = dict(
        n_pad=n_pad, n_win=n_win, gpc=gpc, n_nodes=n_nodes,
        total_passes=total_passes, win_pass_start=win_pass_start,
        seq_len=seq_len, a2h_pad=a2h_pad, n_c=n_c, a2h_dim=a2h_dim,
    )
    data = dict(
        x_t=x_t, src_tiles=src_tiles, dst_tiles=dst_tiles,
        graph_cols=graph_cols, prot_c=prot_c, a2h_t=a2h_t, x_gath=x_gath,
    )
    return meta, data


def _build_program(meta):
    n_pad, n_win, gpc = meta["n_pad"], meta["n_win"], meta["gpc"]
    total_passes = meta["total_passes"]
    wps = meta["win_pass_start"]
    seq_len, a2h_pad = meta["seq_len"], meta["a2h_pad"]
    n_nodes = meta["n_nodes"]
    conv_w = seq_len - 8 + 1
    g_grp = gpc // 4
    f32 = mybir.dt.float32

    nc = bacc.Bacc(None, target_bir_lowering=False, num_devices=N_CORES)

    def par(name, shape, dt=f32):
        return nc.declare_dram_parameter(name, list(shape), dt, isOutput=False)

    x_t_d = par("x_t", (128, n_pad))
    src_d = par("src_tiles", (P, total_passes), mybir.dt.int32)
    dst_d = par("dst_tiles", (P, total_passes), mybir.dt.bfloat16)
    gcol_d = par("graph_cols", (P, n_win))
    prot_d = par("prot_c", (gpc, seq_len))
    a2h_d = par("a2h_t", (a2h_pad, gpc))
    iota_d = par("iota128", (P, P), mybir.dt.bfloat16)
    iota256_d = par("iota256", (P, 256))
    kblk_d = par("kblk", (32, 128))
    convb_d = par("convb_col", (128, 1))
    nreal_d = par("n_real_col", (P, 1))
    xgath_d = par("x_gath", (78, total_passes * P))
    w_d = {}
    for i in range(3):
        din = 78 if i == 0 else HID
        for nm, shp in [
            (f"g{i}W1", (din, HID)), (f"g{i}b1", (HID, 1)),
            (f"g{i}W2", (HID, HID)), (f"g{i}b2", (HID, 1)),
            (f"bn{i}g", (HID, 1)), (f"bn{i}b", (HID, 1)),
        ]:
            w_d[nm] = par(nm, shp)
    for nm, shp in [
        ("ligW", (HID, HID)), ("ligb", (HID, 1)),
        ("protW", (32, HID)), ("protb", (HID, 1)),
        ("a2h1W", (a2h_pad, HID)), ("a2h1b", (HID, 1)),
        ("a2h2W", (HID, HID)), ("a2h2b", (HID, 1)),
        ("c1W00", (HID, HID)), ("c1W10", (HID, HID)), ("c1W20", (HID, HID)),
        ("c1W01", (HID, HID)), ("c1W11", (HID, HID)), ("c1W21", (HID, HID)),
        ("c1ba", (HID, 1)), ("c1bb", (HID, 1)),
        ("c2Wa", (HID, HID)), ("c2Wb", (HID, HID)), ("c2b", (HID, 1)),
        ("outW", (HID, 1)), ("outb", (1, 1)),
    ]:
        w_d[nm] = par(nm, shp)
    out_d = nc.declare_dram_parameter("out", [gpc, 1], f32, isOutput=True)

    bf16 = mybir.dt.bfloat16
    y_nm = nc.dram_tensor("y_nm", [n_pad, HID], bf16)
    y_full = nc.dram_tensor(
        "y_full", [N_CORES * n_pad, HID], bf16, addr_space="Shared"
    )
    stats_in = nc.dram_tensor("stats_in", [P, 2], f32)
    stats_out = nc.dram_tensor(
        "stats_out", [N_CORES * P, 2], f32, addr_space="Shared"
    )
    rg = [list(range(N_CORES))]

    with tile.TileContext(nc) as tc:
        with (
            tc.tile_pool(name="persist", bufs=1) as pp,
            tc.tile_pool(name="work", bufs=2) as wp,
            tc.tile_pool(name="patchp", bufs=1) as patchp,
            tc.tile_pool(name="gpool", bufs=16) as gp,
            tc.tile_pool(name="spool", bufs=16) as sp,
            tc.tile_pool(name="ypool", bufs=6) as yp,
            tc.tile_pool(name="mm128", bufs=3, space="PSUM") as mmp,
            tc.tile_pool(name="h2p", bufs=2, space="PSUM") as h2p,
            tc.tile_pool(name="convp", bufs=1, space="PSUM") as convp,
            tc.tile_pool(name="brp", bufs=1, space="PSUM") as brp,
            tc.tile_pool(name="xgp", bufs=1, space="PSUM") as xgp,
        ):
            # ---- persistent loads ----
            cur = pp.tile([128, n_pad], f32, tag="cur")
            nc.sync.dma_start(cur[:], x_t_d.ap())
            h1_t = pp.tile([128, n_pad], f32)
            src_t = pp.tile([P, total_passes], mybir.dt.int32)
            nc.sync.dma_start(src_t[:], src_d.ap())
            dst_t = pp.tile([P, total_passes], mybir.dt.bfloat16)
            nc.sync.dma_start(dst_t[:], dst_d.ap())
            gcol_t = pp.tile([P, n_win], f32)
            nc.sync.dma_start(gcol_t[:], gcol_d.ap())
            iota_t = pp.tile([P, P], mybir.dt.bfloat16)
            nc.sync.dma_start(iota_t[:], iota_d.ap())
            iota256_t = pp.tile([P, 256], f32)
            nc.sync.dma_start(iota256_t[:], iota256_d.ap())
            kblk_t = pp.tile([32, 128], f32)
            nc.sync.dma_start(kblk_t[:], kblk_d.ap())
            convb_t = pp.tile([128, 1], f32)
            nc.sync.dma_start(convb_t[:], convb_d.ap())
            nrc_t = pp.tile([P, 1], f32)
            nc.sync.dma_start(nrc_t[:], nreal_d.ap())
            for _gi in range(16):
                gz = gp.tile([P, HID], mybir.dt.bfloat16, tag="gt", name=f"gz{_gi}")
                nc.vector.memset(gz[:], 0.0)
            wt = {}
            for k, d in w_d.items():
                if k == "a2h1W":
                    continue
                wt[k] = pp.tile(list(d.shape), f32, tag=f"w_{k}", name=f"w_{k}")
                nc.sync.dma_start(wt[k][:], d.ap())
            ident = pp.tile([P, P], f32)
            make_identity(nc, ident[:])

            # ================= protein conv branch =================
            maxes = pp.tile([128, g_grp], f32)
            for grp in range(g_grp):
                patch = patchp.tile([32, conv_w], f32, tag="patch")
                sl = prot_d.ap()[4 * grp : 4 * grp + 4, :]
                src_ap = bass.AP(
                    sl.tensor, sl.offset, [list(sl.ap[0]), [1, 8], [1, conv_w]]
                )
                nc.sync.dma_start(patch[:], src_ap)
                m2 = wp.tile([128, 2], f32, tag="m2")
                for half, (c0, c1) in enumerate([(0, 512), (512, conv_w)]):
                    cps = convp.tile([128, 512], f32, tag="convps")
                    nc.tensor.matmul(
                        cps[:, : c1 - c0], kblk_t[:], patch[:, c0:c1],
                        start=True, stop=True,
                    )
                    nc.vector.tensor_reduce(
                        m2[:, half : half + 1], cps[:, : c1 - c0],
                        mybir.AxisListType.X, mybir.AluOpType.max,
                    )
                nc.vector.tensor_reduce(
                    maxes[:, grp : grp + 1], m2[:], mybir.AxisListType.X,
                    mybir.AluOpType.max,
                )
            xp_m = pp.tile([128, g_grp], f32)
            nc.scalar.activation(
                xp_m[:], maxes[:], mybir.ActivationFunctionType.Relu,
                bias=convb_t[:], scale=1.0,
            )
            xp_ch = pp.tile([32, gpc], f32)
            xp_ch_v = xp_ch[:].rearrange("c (g gi) -> c g gi", gi=4)
            for gi in range(4):
                nc.sync.dma_start(
                    xp_ch_v[:, :, gi], xp_m[gi * 32 : (gi + 1) * 32, :]
                )
            xp_t = pp.tile([128, gpc], f32)
            pps = brp.tile([128, gpc], f32, tag="brps")
            nc.tensor.matmul(pps[:], wt["protW"][:], xp_ch[:], start=True, stop=True)
            nc.scalar.activation(
                xp_t[:], pps[:], mybir.ActivationFunctionType.Relu,
                bias=wt["protb"][:], scale=1.0,
            )

            # ================= a2h branch =================
            xa1 = pp.tile([128, gpc], f32)
            aps = brp.tile([128, gpc], f32, tag="brps")
            n_chunk = a2h_pad // P
            for ch in range(n_chunk):
                a_tile = wp.tile([P, gpc], f32, tag="a2h")
                nc.sync.dma_start(a_tile[:], a2h_d.ap()[ch * P : (ch + 1) * P, :])
                aw = wp.tile([P, HID], f32, tag="a2hw")
                nc.sync.dma_start(
                    aw[:], w_d["a2h1W"].ap()[ch * P : (ch + 1) * P, :]
                )
                nc.tensor.matmul(
                    aps[:], aw[:], a_tile[:],
                    start=(ch == 0), stop=(ch == n_chunk - 1),
                )
            nc.scalar.activation(
                xa1[:], aps[:], mybir.ActivationFunctionType.Relu,
                bias=wt["a2h1b"][:], scale=1.0,
            )
            xa_t = pp.tile([128, gpc], f32)
            aps2 = brp.tile([128, gpc], f32, tag="brps")
            nc.tensor.matmul(aps2[:], wt["a2h2W"][:], xa1[:], start=True, stop=True)
            nc.scalar.activation(
                xa_t[:], aps2[:], mybir.ActivationFunctionType.Relu,
                bias=wt["a2h2b"][:], scale=1.0,
            )

            # ================= GIN layers =================
            # xpad_col: value of pad-node activation columns (constant per
            # feature). Starts at zero (host zero-pads x).
            xpad = pp.tile([128, 1], f32)
            nc.vector.memset(xpad[:], 0.0)
            for li in range(3):
                kdim = 78 if li == 0 else HID
                W1, W2 = wt[f"g{li}W1"], wt[f"g{li}W2"]
                b1, b2 = wt[f"g{li}b1"], wt[f"g{li}b2"]
                # ---- y node-major -> DRAM, then AllGather (layers 1,2) ----
                if li > 0:
                    for w in range(n_win):
                        yps = mmp.tile([P, HID], f32, tag="mm128")
                        nc.tensor.matmul(
                            yps[:], cur[:kdim, w * P : (w + 1) * P],
                            W1[:kdim, :], start=True, stop=True,
                        )
                        y_sb = yp.tile([P, HID], mybir.dt.bfloat16, tag="ysb")
                        nc.vector.tensor_copy(y_sb[:], yps[:])
                        nc.sync.dma_start(y_nm[w * P : (w + 1) * P, :], y_sb[:])
                    nc.gpsimd.collective_compute(
                        "AllGather", mybir.AluOpType.bypass, replica_groups=rg,
                        ins=[y_nm.ap().opt()], outs=[y_full.ap().opt()],
                    )
                # ---- aggregation ----
                g_tiles = {}
                xg_blks = {}
                for w in range(n_win):
                    agg = mmp.tile([128, P], f32, tag="mm128")
                    nc.tensor.matmul(
                        agg[:], W1[:kdim, :], cur[:kdim, w * P : (w + 1) * P],
                        start=True, stop=False,
                    )
                    p0, p1 = int(wps[w]), int(wps[w + 1])
                    for pi in range(p0, p1):
                        ck = pi // GCHUNK
                        if ck not in g_tiles:
                            gt = gp.tile([P, HID], mybir.dt.bfloat16, tag="gt")
                            if li == 0:
                                blk8 = ck // 8
                                if blk8 not in xg_blks:
                                    xg_sl = wp.tile(
                                        [78, 8 * P], f32, tag="xgsl"
                                    )
                                    lo = blk8 * 8 * P
                                    hi = min(lo + 8 * P, total_passes * P)
                                    nc.sync.dma_start(
                                        xg_sl[:, : hi - lo],
                                        xgath_d.ap()[:, lo:hi],
                                    )
                                    xg_blks = {blk8: xg_sl}
                                xs = xg_blks[blk8]
                                o = (ck % 8) * P
                                gps_t = h2p.tile([128, 512], f32, tag="h2ps")
                                nc.tensor.matmul(
                                    gps_t[:, :HID], xs[:, o : o + P],
                                    W1[:kdim, :], start=True, stop=True,
                                )
                                nc.scalar.copy(gt[:], gps_t[:, :HID])
                            else:
                                nc.gpsimd.indirect_dma_start(
                                    out=gt[:],
                                    out_offset=None,
                                    in_=y_full.ap(),
                                    in_offset=bass.IndirectOffsetOnAxis(
                                        ap=src_t[:, ck : ck + 1],
                                        axis=0,
                                    ),
                                    bounds_check=N_CORES * n_pad - 1,
                                    oob_is_err=False,
                                )
                            g_tiles = {ck: gt}
                        s_tile = sp.tile([P, P], mybir.dt.bfloat16, tag="s1h")
                        nc.vector.tensor_tensor(
                            out=s_tile[:],
                            in0=iota_t[:],
                            in1=dst_t[:, pi : pi + 1].to_broadcast([P, P]),
                            op=mybir.AluOpType.is_equal,
                        )
                        nc.tensor.matmul(
                            agg[:], g_tiles[ck][:], s_tile[:],
                            start=False, stop=(pi == p1 - 1),
                        )
                    nc.scalar.activation(
                        h1_t[:, w * P : (w + 1) * P], agg[:],
                        mybir.ActivationFunctionType.Relu,
                        bias=b1[:], scale=1.0,
                    )
                # pad-column constants: ypad = W1^T xpad ; h1pad = relu(+b1)
                ypps = mmp.tile([128, 1], f32, tag="mm128")
                nc.tensor.matmul(
                    ypps[:, 0:1], W1[:kdim, :], xpad[:kdim, 0:1],
                    start=True, stop=True,
                )
                h1pad = wp.tile([128, 1], f32, tag="h1pad")
                nc.scalar.activation(
                    h1pad[:], ypps[:, 0:1],
                    mybir.ActivationFunctionType.Relu, bias=b1[:], scale=1.0,
                )
                vpps = mmp.tile([128, 1], f32, tag="mm128")
                nc.tensor.matmul(
                    vpps[:, 0:1], W2[:], h1pad[:], start=True, stop=True
                )
                vpad = wp.tile([128, 1], f32, tag="vpad")
                nc.vector.tensor_copy(vpad[:], vpps[:, 0:1])
                # ---- W2 pass 1: partial sums / sumsq ----
                n_big = (n_pad + 511) // 512
                parts = wp.tile([128, 2 * n_big], f32, tag="parts")
                for b in range(n_big):
                    c0, c1 = b * 512, min((b + 1) * 512, n_pad)
                    wl = c1 - c0
                    h2ps = h2p.tile([128, 512], f32, tag="h2ps")
                    nc.tensor.matmul(
                        h2ps[:, :wl], W2[:], h1_t[:, c0:c1],
                        start=True, stop=True,
                    )
                    sq = wp.tile([128, 512], f32, tag="sq")
                    nc.scalar.activation(
                        sq[:, :wl], h2ps[:, :wl],
                        mybir.ActivationFunctionType.Square,
                    )
                    nc.vector.tensor_reduce(
                        parts[:, 2 * b : 2 * b + 1], h2ps[:, :wl],
                        mybir.AxisListType.X, mybir.AluOpType.add,
                    )
                    nc.vector.tensor_reduce(
                        parts[:, 2 * b + 1 : 2 * b + 2], sq[:, :wl],
                        mybir.AxisListType.X, mybir.AluOpType.add,
                    )
                stats = wp.tile([128, 2], f32, tag="stats")
                pv = parts[:].rearrange("p (b s) -> p s b", s=2)
                nc.vector.tensor_reduce(
                    stats[:, 0:1], pv[:, 0, :], mybir.AxisListType.X,
                    mybir.AluOpType.add,
                )
                nc.vector.tensor_reduce(
                    stats[:, 1:2], pv[:, 1, :], mybir.AxisListType.X,
                    mybir.AluOpType.add,
                )
                # subtract pad-column contribution: ndead * [v, v^2]
                ndead = wp.tile([128, 1], f32, tag="ndead")
                nc.vector.tensor_scalar(
                    out=ndead[:], in0=nrc_t[:], scalar1=-1.0,
                    scalar2=float(n_pad), op0=mybir.AluOpType.mult,
                    op1=mybir.AluOpType.add,
                )
                vsq = wp.tile([128, 1], f32, tag="vsq")
                nc.vector.tensor_tensor(
                    out=vsq[:], in0=vpad[:], in1=vpad[:],
                    op=mybir.AluOpType.mult,
                )
                corr = wp.tile([128, 2], f32, tag="corr")
                nc.vector.tensor_tensor(
                    out=corr[:, 0:1], in0=vpad[:], in1=ndead[:],
                    op=mybir.AluOpType.mult,
                )
                nc.vector.tensor_tensor(
                    out=corr[:, 1:2], in0=vsq[:], in1=ndead[:],
                    op=mybir.AluOpType.mult,
                )
                nc.vector.tensor_tensor(
                    out=stats[:], in0=stats[:], in1=corr[:],
                    op=mybir.AluOpType.subtract,
                )
                nc.sync.dma_start(stats_in[:, :], stats[:])
                nc.gpsimd.collective_compute(
                    "AllGather", mybir.AluOpType.bypass, replica_groups=rg,
                    ins=[stats_in.ap().opt()], outs=[stats_out.ap().opt()],
                )
                allst = wp.tile([128, N_CORES, 2], f32, tag="allst")
                nc.sync.dma_start(
                    allst[:], stats_out.ap().rearrange("(c p) s -> p c s", p=P)
                )
                tot = wp.tile([128, 2], f32, tag="tot")
                av = allst[:].rearrange("p c s -> p s c")
                nc.vector.tensor_reduce(
                    tot[:, 0:1], av[:, 0, :], mybir.AxisListType.X,
                    mybir.AluOpType.add,
                )
                nc.vector.tensor_reduce(
                    tot[:, 1:2], av[:, 1, :], mybir.AxisListType.X,
                    mybir.AluOpType.add,
                )
                inv_n = 1.0 / float(n_nodes)
                mu = wp.tile([128, 1], f32, tag="mu")
                nc.vector.tensor_scalar(
                    out=mu[:], in0=tot[:, 0:1], scalar1=inv_n, scalar2=None,
                    op0=mybir.AluOpType.mult,
                )
                var = wp.tile([128, 1], f32, tag="var")
                nc.vector.tensor_scalar(
                    out=var[:], in0=tot[:, 1:2], scalar1=inv_n, scalar2=None,
                    op0=mybir.AluOpType.mult,
                )
                musq = wp.tile([128, 1], f32, tag="musq")
                nc.vector.tensor_tensor(
                    out=musq[:], in0=mu[:], in1=mu[:], op=mybir.AluOpType.mult
                )
                nc.vector.tensor_tensor(
                    out=var[:], in0=var[:], in1=musq[:],
                    op=mybir.AluOpType.subtract,
                )
                nc.vector.tensor_scalar(
                    out=var[:], in0=var[:], scalar1=BN_EPS, scalar2=None,
                    op0=mybir.AluOpType.add,
                )
                sd = wp.tile([128, 1], f32, tag="sd")
                nc.scalar.sqrt(sd[:], var[:])
                inv_sd = wp.tile([128, 1], f32, tag="invsd")
                nc.vector.reciprocal(inv_sd[:], sd[:])
                A = wp.tile([128, 1], f32, tag="A")
                nc.vector.tensor_tensor(
                    out=A[:], in0=inv_sd[:], in1=wt[f"bn{li}g"][:],
                    op=mybir.AluOpType.mult,
                )
                negmuA = wp.tile([128, 1], f32, tag="negmuA")
                nc.vector.tensor_tensor(
                    out=negmuA[:], in0=mu[:], in1=A[:], op=mybir.AluOpType.mult
                )
                B = wp.tile([128, 1], f32, tag="B")
                nc.vector.tensor_tensor(
                    out=B[:], in0=wt[f"bn{li}b"][:], in1=negmuA[:],
                    op=mybir.AluOpType.subtract,
                )
                # ---- W2 pass 2 + BN + relu (in place: cur is dead) ----
                nxt = cur
                for b in range(n_big):
                    c0, c1 = b * 512, min((b + 1) * 512, n_pad)
                    wl = c1 - c0
                    h2ps = h2p.tile([128, 512], f32, tag="h2ps")
                    nc.tensor.matmul(
                        h2ps[:, :wl], W2[:], h1_t[:, c0:c1],
                        start=True, stop=True,
                    )
                    nc.scalar.activation(
                        nxt[:, c0:c1], h2ps[:, :wl],
                        mybir.ActivationFunctionType.Relu,
                        bias=B[:], scale=A[:],
                    )
                # update pad-column constant for next layer (in place)
                nc.scalar.activation(
                    xpad[:], vpad[:], mybir.ActivationFunctionType.Relu,
                    bias=B[:], scale=A[:],
                )
                cur = nxt

            # ================= pooling + lig MLP =================
            xg_ps = xgp.tile([128, gpc], f32, tag="xgps")
            for w in range(n_win):
                tps = mmp.tile([P, P], f32, tag="mm128")
                nc.tensor.transpose(tps[:], cur[:, w * P : (w + 1) * P], ident[:])
                x_nm = wp.tile([P, P], f32, tag="xnm")
                nc.vector.tensor_copy(x_nm[:], tps[:])
                pool1h = sp.tile([P, 256], f32, tag="pool1h")
                nc.vector.tensor_tensor(
                    out=pool1h[:],
                    in0=iota256_t[:],
                    in1=gcol_t[:, w : w + 1].to_broadcast([P, 256]),
                    op=mybir.AluOpType.is_equal,
                )
                nc.tensor.matmul(
                    xg_ps[:], x_nm[:], pool1h[:, :gpc],
                    start=(w == 0), stop=(w == n_win - 1),
                )
            xg_sb = pp.tile([128, gpc], f32)
            nc.vector.tensor_copy(xg_sb[:], xg_ps[:])
            lps = brp.tile([128, gpc], f32, tag="brps")
            nc.tensor.matmul(lps[:], wt["ligW"][:], xg_sb[:], start=True, stop=True)
            xg_t = pp.tile([128, gpc], f32)
            nc.scalar.activation(
                xg_t[:], lps[:], mybir.ActivationFunctionType.Relu,
                bias=wt["ligb"][:], scale=1.0,
            )

            # ================= head =================
            branches = [xg_t, xp_t, xa_t]
            xc1a = pp.tile([128, gpc], f32)
            xc1b = pp.tile([128, gpc], f32)
            for h, (xc1h, bkey) in enumerate(
                [(xc1a, "c1ba"), (xc1b, "c1bb")]
            ):
                hps = brp.tile([128, gpc], f32, tag="brps")
                for blk in range(3):
                    nc.tensor.matmul(
                        hps[:], wt[f"c1W{blk}{h}"][:],
                        branches[blk][:],
                        start=(blk == 0), stop=(blk == 2),
                    )
                nc.scalar.activation(
                    xc1h[:], hps[:], mybir.ActivationFunctionType.Relu,
                    bias=wt[bkey][:], scale=1.0,
                )
            c2ps = brp.tile([128, gpc], f32, tag="brps")
            nc.tensor.matmul(c2ps[:], wt["c2Wa"][:], xc1a[:], start=True, stop=False)
            nc.tensor.matmul(c2ps[:], wt["c2Wb"][:], xc1b[:], start=False, stop=True)
            xc2 = pp.tile([128, gpc], f32)
            nc.scalar.activation(
                xc2[:], c2ps[:], mybir.ActivationFunctionType.Relu,
                bias=wt["c2b"][:], scale=1.0,
            )
            ops = brp.tile([128, gpc], f32, tag="brps")
            nc.tensor.matmul(ops[:1, :], wt["outW"][:], xc2[:], start=True, stop=True)
            ovec = wp.tile([1, gpc], f32, tag="ovec")
            nc.scalar.activation(
                ovec[:], ops[:1, :], mybir.ActivationFunctionType.Identity,
                bias=wt["outb"][:], scale=1.0,
            )
            nc.sync.dma_start(out_d.ap(), ovec[:])

    return nc


def kernel(**inputs):
    meta, data = _build_host_data(
        inputs["x_ligand"], inputs["protein_seq"], inputs["a2h"],
        inputs["edge_index"], inputs["batch_ligand"],
    )
    gpc = meta["gpc"]
    key = (meta["n_pad"], meta["total_passes"], gpc, meta["seq_len"])
    if key not in _cache:
        nc = _build_program(meta)
        nc.compile()
        _cache[key] = nc
    nc = _cache[key]

    f32 = np.float32
    bf = np.dtype("bfloat16")
    iota128 = np.tile(np.arange(P, dtype=f32), (P, 1)).astype(bf)
    iota256 = np.tile(np.arange(256, dtype=f32), (P, 1))
    conv_k = np.asarray(inputs["conv_k"], f32)
    kblk = np.zeros((32, 128), f32)
    for g in range(4):
        kblk[g * 8 : (g + 1) * 8, g * 32 : (g + 1) * 32] = conv_k[:, 0, :].T
    convb_col = np.tile(np.asarray(inputs["conv_b"], f32), 4).reshape(128, 1)

    shared = {
        "iota128": iota128, "iota256": iota256, "kblk": kblk,
        "convb_col": convb_col,
    }
    col = lambda v: np.asarray(v, f32).reshape(-1, 1)
    for i in range(3):
        shared[f"g{i}W1"] = np.asarray(inputs[f"gin{i}_W1"], f32)
        shared[f"g{i}b1"] = col(inputs[f"gin{i}_b1"])
        shared[f"g{i}W2"] = np.asarray(inputs[f"gin{i}_W2"], f32)
        shared[f"g{i}b2"] = col(inputs[f"gin{i}_b2"])
        shared[f"bn{i}g"] = col(inputs[f"bn{i}_g"])
        shared[f"bn{i}b"] = col(inputs[f"bn{i}_b"])
    shared["ligW"] = np.asarray(inputs["lig_W"], f32)
    shared["ligb"] = col(inputs["lig_b"])
    shared["protW"] = np.asarray(inputs["prot_W"], f32)
    shared["protb"] = col(inputs["prot_b"])
    a2h1W = np.zeros((meta["a2h_pad"], HID), f32)
    a2h1W[: meta["a2h_dim"]] = np.asarray(inputs["a2h1_W"], f32)
    shared["a2h1W"] = a2h1W
    shared["a2h1b"] = col(inputs["a2h1_b"])
    shared["a2h2W"] = np.asarray(inputs["a2h2_W"], f32)
    shared["a2h2b"] = col(inputs["a2h2_b"])
    c1W = np.asarray(inputs["c1_W"], f32)
    for blk in range(3):
        for h in range(2):
            shared[f"c1W{blk}{h}"] = np.ascontiguousarray(
                c1W[blk * HID : (blk + 1) * HID, h * HID : (h + 1) * HID]
            )
    c1b = np.asarray(inputs["c1_b"], f32)
    shared["c1ba"] = col(c1b[:HID])
    shared["c1bb"] = col(c1b[HID:])
    c2W = np.asarray(inputs["c2_W"], f32)
    shared["c2Wa"] = np.ascontiguousarray(c2W[:HID, :])
    shared["c2Wb"] = np.ascontiguousarray(c2W[HID:, :])
    shared["c2b"] = col(inputs["c2_b"])
    shared["outW"] = np.asarray(inputs["out_W"], f32)
    shared["outb"] = np.asarray(inputs["out_b"], f32).reshape(1, 1)

    in_maps = []
    for c in range(N_CORES):
        m = dict(shared)
        m["x_t"] = data["x_t"][c]
        m["src_tiles"] = data["src_tiles"][c]
        m["dst_tiles"] = data["dst_tiles"][c].astype(np.dtype("bfloat16"))
        m["graph_cols"] = data["graph_cols"][c]
        m["prot_c"] = data["prot_c"][c]
        m["a2h_t"] = data["a2h_t"][c]
        m["n_real_col"] = np.full((P, 1), float(meta["n_c"][c]), f32)
        m["x_gath"] = data["x_gath"][c]
        in_maps.append(m)

    res = run_bass_kernel_spmd(nc, in_maps, core_ids=list(range(N_CORES)))
    outs = [
        np.asarray(res.results[c]["out"]).reshape(gpc, 1)
        for c in range(N_CORES)
    ]
    return np.concatenate(outs, axis=0).astype(np.float32)


# revision 16
# speedup vs baseline: 1.0782x; 1.0450x over previous
"""Trainium2 8-core Bass kernel for A2HNet (GIN message passing + branches).

Self-contained: computes all sharding/index structures from the inputs at
call time, builds one SPMD Bass program, runs it on cores 0-7, and gathers
the full (2048, 1) output.

Sharding: graphs (and their nodes) are block-partitioned over the 8 cores.
Each GIN layer computes y = x @ W1 locally, AllGathers y (node-major) into a
replicated table, then each core aggregates its own nodes' incoming edges
with an indirect-DMA row gather feeding one-hot matmuls that accumulate
y[dst] + sum_e y[src_e] directly in PSUM. BatchNorm batch stats are
AllGathered as per-core partial sums. The protein-conv / a2h / head branches
are computed per-core on the local 256 graphs.
"""
import numpy as np

from concourse import bacc, bass, mybir, tile
from concourse.bass_utils import run_bass_kernel_spmd
from concourse.masks import make_identity

N_CORES = 8
HID = 128
BN_EPS = 1e-5
P = 128
GCHUNK = 1  # passes per indirect gather call (128 rows)

_cache = {}


def _build_host_data(x_ligand, protein_seq, a2h, edge_index, batch_ligand):
    n_nodes = x_ligand.shape[0]
    n_graphs = a2h.shape[0]
    gpc = n_graphs // N_CORES

    batch = np.asarray(batch_ligand).astype(np.int64)
    src_all = np.asarray(edge_index)[0].astype(np.int64)
    dst_all = np.asarray(edge_index)[1].astype(np.int64)

    node_core = batch // gpc
    core_starts = np.searchsorted(node_core, np.arange(N_CORES))
    core_ends = np.searchsorted(node_core, np.arange(N_CORES), side="right")
    n_c = core_ends - core_starts
    n_pad = int(np.ceil(n_c.max() / P) * P)
    n_win = n_pad // P

    # relabel nodes per core by descending in-degree: equalizes per-window
    # edge counts across cores, shrinking max-over-cores pass padding
    deg = np.bincount(dst_all, minlength=n_nodes)
    local_old = np.arange(n_nodes) - core_starts[node_core]
    local_idx = np.empty(n_nodes, np.int64)
    perms = []
    for c in range(N_CORES):
        s_, e_ = int(core_starts[c]), int(core_ends[c])
        perm = np.argsort(-deg[s_:e_], kind="stable")
        invp = np.argsort(perm, kind="stable")
        local_idx[s_:e_] = invp
        perms.append(perm)
    gid = node_core * n_pad + local_idx

    edge_core = node_core[dst_all]
    counts = np.zeros((N_CORES, n_win), np.int64)
    per_core_edges = []
    for c in range(N_CORES):
        m = edge_core == c
        e_src = src_all[m]
        e_dst_loc = local_idx[dst_all[m]]
        order = np.argsort(e_dst_loc, kind="stable")
        e_src, e_dst_loc = e_src[order], e_dst_loc[order]
        counts[c] = np.bincount(e_dst_loc // P, minlength=n_win)
        per_core_edges.append((e_src, e_dst_loc))

    passes_w = np.maximum(1, np.ceil(counts.max(axis=0) / P).astype(np.int64))
    total_passes = int(passes_w.sum())
    rem = (-total_passes) % GCHUNK
    passes_w[-1] += rem
    total_passes += rem
    win_pass_start = np.zeros(n_win + 1, np.int64)
    win_pass_start[1:] = np.cumsum(passes_w)

    src_tiles = np.full((N_CORES, P, total_passes), 1 << 22, np.int32)
    dst_tiles = np.full((N_CORES, P, total_passes), 300.0, np.float32)
    for c in range(N_CORES):
        e_src, e_dst_loc = per_core_edges[c]
        w_of = e_dst_loc // P
        off_in_w = np.arange(len(e_src)) - np.searchsorted(w_of, w_of)
        slot = win_pass_start[w_of] * P + off_in_w
        src_tiles[c, slot % P, slot // P] = gid[e_src]
        dst_tiles[c, slot % P, slot // P] = (e_dst_loc % P).astype(np.float32)

    x_t = np.zeros((N_CORES, 128, n_pad), np.float32)
    graph_cols = np.full((N_CORES, P, n_win), 300.0, np.float32)
    xl = np.asarray(x_ligand).astype(np.float32)
    for c in range(N_CORES):
        s, e = int(core_starts[c]), int(core_ends[c])
        x_t[c, :78, : e - s] = xl[s + perms[c]].T
        col = np.full(n_pad, 300.0, np.float32)
        col[: e - s] = (batch[s + perms[c]] - c * gpc).astype(np.float32)
        graph_cols[c] = col.reshape(n_win, P).T

    prot = np.asarray(protein_seq).astype(np.float32).reshape(n_graphs, -1)
    seq_len = prot.shape[1]
    a2h_flat = np.asarray(a2h).astype(np.float32).reshape(n_graphs, -1)
    a2h_dim = a2h_flat.shape[1]
    a2h_pad = int(np.ceil(a2h_dim / P) * P)
    prot_c = np.ascontiguousarray(prot.reshape(N_CORES, gpc, seq_len))
    a2h_t = np.zeros((N_CORES, a2h_pad, gpc), np.float32)
    for c in range(N_CORES):
        a2h_t[c, :a2h_dim, :] = a2h_flat[c * gpc : (c + 1) * gpc].T

    x_gath = np.zeros((N_CORES, 78, total_passes * P), np.float32)
    inv = np.full(N_CORES * n_pad, -1, np.int64)
    for c in range(N_CORES):
        s_, e_ = int(core_starts[c]), int(core_ends[c])
        inv[c * n_pad : c * n_pad + (e_ - s_)] = s_ + perms[c]
    for c in range(N_CORES):
        st = src_tiles[c]  # (P, total_passes), gid or 1<<22
        flat = st.T.reshape(-1)  # slot (pi, p) at pi*P + p
        valid = flat < N_CORES * n_pad
        rows = inv[flat[valid]]
        cols = np.where(valid)[0]
        x_gath[c][:, cols] = xl[rows].T
    meta = dict(
        n_pad=n_pad, n_win=n_win, gpc=gpc, n_nodes=n_nodes,
        total_passes=total_passes, win_pass_start=win_pass_start,
        seq_len=seq_len, a2h_pad=a2h_pad, n_c=n_c, a2h_dim=a2h_dim,
    )
    data = dict(
        x_t=x_t, src_tiles=src_tiles, dst_tiles=dst_tiles,
        graph_cols=graph_cols, prot_c=prot_c, a2h_t=a2h_t, x_gath=x_gath,
    )
    return meta, data


def _build_program(meta):
    n_pad, n_win, gpc = meta["n_pad"], meta["n_win"], meta["gpc"]
    total_passes = meta["total_passes"]
    wps = meta["win_pass_start"]
    seq_len, a2h_pad = meta["seq_len"], meta["a2h_pad"]
    n_nodes = meta["n_nodes"]
    conv_w = seq_len - 8 + 1
    g_grp = gpc // 4
    f32 = mybir.dt.float32

    nc = bacc.Bacc(None, target_bir_lowering=False, num_devices=N_CORES)

    def par(name, shape, dt=f32):
        return nc.declare_dram_parameter(name, list(shape), dt, isOutput=False)

    x_t_d = par("x_t", (128, n_pad))
    src_d = par("src_tiles", (P, total_passes), mybir.dt.int32)
    dst_d = par("dst_tiles", (P, total_passes), mybir.dt.bfloat16)
    gcol_d = par("graph_cols", (P, n_win))
    prot_d = par("prot_c", (gpc, seq_len))
    a2h_d = par("a2h_t", (a2h_pad, gpc))
    iota_d = par("iota128", (P, P), mybir.dt.bfloat16)
    iota256_d = par("iota256", (P, 256))
    kblk_d = par("kblk", (32, 128))
    convb_d = par("convb_col", (128, 1))
    nreal_d = par("n_real_col", (P, 1))
    xgath_d = par("x_gath", (78, total_passes * P), mybir.dt.bfloat16)
    g0w1bf_d = par("g0W1bf", (78, HID), mybir.dt.bfloat16)
    w_d = {}
    for i in range(3):
        din = 78 if i == 0 else HID
        for nm, shp in [
            (f"g{i}W1", (din, HID)), (f"g{i}b1", (HID, 1)),
            (f"g{i}W2", (HID, HID)), (f"g{i}b2", (HID, 1)),
            (f"bn{i}g", (HID, 1)), (f"bn{i}b", (HID, 1)),
        ]:
            w_d[nm] = par(nm, shp)
    for nm, shp in [
        ("ligW", (HID, HID)), ("ligb", (HID, 1)),
        ("protW", (32, HID)), ("protb", (HID, 1)),
        ("a2h1W", (a2h_pad, HID)), ("a2h1b", (HID, 1)),
        ("a2h2W", (HID, HID)), ("a2h2b", (HID, 1)),
        ("c1W00", (HID, HID)), ("c1W10", (HID, HID)), ("c1W20", (HID, HID)),
        ("c1W01", (HID, HID)), ("c1W11", (HID, HID)), ("c1W21", (HID, HID)),
        ("c1ba", (HID, 1)), ("c1bb", (HID, 1)),
        ("c2Wa", (HID, HID)), ("c2Wb", (HID, HID)), ("c2b", (HID, 1)),
        ("outW", (HID, 1)), ("outb", (1, 1)),
    ]:
        w_d[nm] = par(nm, shp)
    out_d = nc.declare_dram_parameter("out", [gpc, 1], f32, isOutput=True)

    bf16 = mybir.dt.bfloat16
    y_nm = nc.dram_tensor("y_nm", [n_pad, HID], bf16)
    y_full = nc.dram_tensor(
        "y_full", [N_CORES * n_pad, HID], bf16, addr_space="Shared"
    )
    stats_in = nc.dram_tensor("stats_in", [P, 2], f32)
    stats_out = nc.dram_tensor(
        "stats_out", [N_CORES * P, 2], f32, addr_space="Shared"
    )
    rg = [list(range(N_CORES))]

    with tile.TileContext(nc) as tc:
        with (
            tc.tile_pool(name="persist", bufs=1) as pp,
            tc.tile_pool(name="work", bufs=2) as wp,
            tc.tile_pool(name="patchp", bufs=1) as patchp,
            tc.tile_pool(name="gpool", bufs=16) as gp,
            tc.tile_pool(name="spool", bufs=16) as sp,
            tc.tile_pool(name="ypool", bufs=6) as yp,
            tc.tile_pool(name="mm128", bufs=3, space="PSUM") as mmp,
            tc.tile_pool(name="h2p", bufs=2, space="PSUM") as h2p,
            tc.tile_pool(name="convp", bufs=1, space="PSUM") as convp,
            tc.tile_pool(name="brp", bufs=1, space="PSUM") as brp,
            tc.tile_pool(name="xgp", bufs=1, space="PSUM") as xgp,
        ):
            # ---- persistent loads ----
            cur = pp.tile([128, n_pad], f32, tag="cur")
            nc.sync.dma_start(cur[:], x_t_d.ap())
            h1_t = pp.tile([128, n_pad], f32)
            src_t = pp.tile([P, total_passes], mybir.dt.int32)
            nc.sync.dma_start(src_t[:], src_d.ap())
            dst_t = pp.tile([P, total_passes], mybir.dt.bfloat16)
            nc.sync.dma_start(dst_t[:], dst_d.ap())
            gcol_t = pp.tile([P, n_win], f32)
            nc.sync.dma_start(gcol_t[:], gcol_d.ap())
            iota_t = pp.tile([P, P], mybir.dt.bfloat16)
            nc.sync.dma_start(iota_t[:], iota_d.ap())
            iota256_t = pp.tile([P, 256], f32)
            nc.sync.dma_start(iota256_t[:], iota256_d.ap())
            kblk_t = pp.tile([32, 128], f32)
            nc.sync.dma_start(kblk_t[:], kblk_d.ap())
            convb_t = pp.tile([128, 1], f32)
            nc.sync.dma_start(convb_t[:], convb_d.ap())
            nrc_t = pp.tile([P, 1], f32)
            nc.sync.dma_start(nrc_t[:], nreal_d.ap())
            for _gi in range(16):
                gz = gp.tile([P, HID], mybir.dt.bfloat16, tag="gt", name=f"gz{_gi}")
                nc.vector.memset(gz[:], 0.0)
            wt = {}
            for k, d in w_d.items():
                if k == "a2h1W":
                    continue
                wt[k] = pp.tile(list(d.shape), f32, tag=f"w_{k}", name=f"w_{k}")
                nc.sync.dma_start(wt[k][:], d.ap())
            g0w1bf = pp.tile([78, HID], mybir.dt.bfloat16)
            nc.sync.dma_start(g0w1bf[:], g0w1bf_d.ap())
            ident = pp.tile([P, P], f32)
            make_identity(nc, ident[:])

            # ================= protein conv branch =================
            maxes = pp.tile([128, g_grp], f32)
            for grp in range(g_grp):
                patch = patchp.tile([32, conv_w], f32, tag="patch")
                sl = prot_d.ap()[4 * grp : 4 * grp + 4, :]
                src_ap = bass.AP(
                    sl.tensor, sl.offset, [list(sl.ap[0]), [1, 8], [1, conv_w]]
                )
                nc.sync.dma_start(patch[:], src_ap)
                m2 = wp.tile([128, 2], f32, tag="m2")
                for half, (c0, c1) in enumerate([(0, 512), (512, conv_w)]):
                    cps = convp.tile([128, 512], f32, tag="convps")
                    nc.tensor.matmul(
                        cps[:, : c1 - c0], kblk_t[:], patch[:, c0:c1],
                        start=True, stop=True,
                    )
                    nc.vector.tensor_reduce(
                        m2[:, half : half + 1], cps[:, : c1 - c0],
                        mybir.AxisListType.X, mybir.AluOpType.max,
                    )
                nc.vector.tensor_reduce(
                    maxes[:, grp : grp + 1], m2[:], mybir.AxisListType.X,
                    mybir.AluOpType.max,
                )
            xp_m = pp.tile([128, g_grp], f32)
            nc.scalar.activation(
                xp_m[:], maxes[:], mybir.ActivationFunctionType.Relu,
                bias=convb_t[:], scale=1.0,
            )
            xp_ch = pp.tile([32, gpc], f32)
            xp_ch_v = xp_ch[:].rearrange("c (g gi) -> c g gi", gi=4)
            for gi in range(4):
                nc.sync.dma_start(
                    xp_ch_v[:, :, gi], xp_m[gi * 32 : (gi + 1) * 32, :]
                )
            xp_t = pp.tile([128, gpc], f32)
            pps = brp.tile([128, gpc], f32, tag="brps")
            nc.tensor.matmul(pps[:], wt["protW"][:], xp_ch[:], start=True, stop=True)
            nc.scalar.activation(
                xp_t[:], pps[:], mybir.ActivationFunctionType.Relu,
                bias=wt["protb"][:], scale=1.0,
            )

            # ================= a2h branch =================
            xa1 = pp.tile([128, gpc], f32)
            aps = brp.tile([128, gpc], f32, tag="brps")
            n_chunk = a2h_pad // P
            for ch in range(n_chunk):
                a_tile = wp.tile([P, gpc], f32, tag="a2h")
                nc.sync.dma_start(a_tile[:], a2h_d.ap()[ch * P : (ch + 1) * P, :])
                aw = wp.tile([P, HID], f32, tag="a2hw")
                nc.sync.dma_start(
                    aw[:], w_d["a2h1W"].ap()[ch * P : (ch + 1) * P, :]
                )
                nc.tensor.matmul(
                    aps[:], aw[:], a_tile[:],
                    start=(ch == 0), stop=(ch == n_chunk - 1),
                )
            nc.scalar.activation(
                xa1[:], aps[:], mybir.ActivationFunctionType.Relu,
                bias=wt["a2h1b"][:], scale=1.0,
            )
            xa_t = pp.tile([128, gpc], f32)
            aps2 = brp.tile([128, gpc], f32, tag="brps")
            nc.tensor.matmul(aps2[:], wt["a2h2W"][:], xa1[:], start=True, stop=True)
            nc.scalar.activation(
                xa_t[:], aps2[:], mybir.ActivationFunctionType.Relu,
                bias=wt["a2h2b"][:], scale=1.0,
            )

            # ================= GIN layers =================
            # xpad_col: value of pad-node activation columns (constant per
            # feature). Starts at zero (host zero-pads x).
            xpad = pp.tile([128, 1], f32)
            nc.vector.memset(xpad[:], 0.0)
            for li in range(3):
                kdim = 78 if li == 0 else HID
                W1, W2 = wt[f"g{li}W1"], wt[f"g{li}W2"]
                b1, b2 = wt[f"g{li}b1"], wt[f"g{li}b2"]
                # ---- y node-major -> DRAM, then AllGather (layers 1,2) ----
                if li > 0:
                    for w in range(n_win):
                        yps = mmp.tile([P, HID], f32, tag="mm128")
                        nc.tensor.matmul(
                            yps[:], cur[:kdim, w * P : (w + 1) * P],
                            W1[:kdim, :], start=True, stop=True,
                        )
                        y_sb = yp.tile([P, HID], mybir.dt.bfloat16, tag="ysb")
                        nc.vector.tensor_copy(y_sb[:], yps[:])
                        nc.sync.dma_start(y_nm[w * P : (w + 1) * P, :], y_sb[:])
                    nc.gpsimd.collective_compute(
                        "AllGather", mybir.AluOpType.bypass, replica_groups=rg,
                        ins=[y_nm.ap().opt()], outs=[y_full.ap().opt()],
                    )
                # ---- aggregation ----
                g_tiles = {}
                xg_blks = {}
                for w in range(n_win):
                    agg = mmp.tile([128, P], f32, tag="mm128")
                    nc.tensor.matmul(
                        agg[:], W1[:kdim, :], cur[:kdim, w * P : (w + 1) * P],
                        start=True, stop=False,
                    )
                    p0, p1 = int(wps[w]), int(wps[w + 1])
                    for pi in range(p0, p1):
                        ck = pi // GCHUNK
                        if ck not in g_tiles:
                            gt = gp.tile([P, HID], mybir.dt.bfloat16, tag="gt")
                            if li == 0:
                                blk8 = ck // 8
                                if blk8 not in xg_blks:
                                    xg_sl = wp.tile(
                                        [78, 8 * P], mybir.dt.bfloat16,
                                        tag="xgsl"
                                    )
                                    lo = blk8 * 8 * P
                                    hi = min(lo + 8 * P, total_passes * P)
                                    nc.sync.dma_start(
                                        xg_sl[:, : hi - lo],
                                        xgath_d.ap()[:, lo:hi],
                                    )
                                    xg_blks = {blk8: xg_sl}
                                xs = xg_blks[blk8]
                                o = (ck % 8) * P
                                gps_t = h2p.tile([128, 512], f32, tag="h2ps")
                                nc.tensor.matmul(
                                    gps_t[:, :HID], xs[:, o : o + P],
                                    g0w1bf[:], start=True, stop=True,
                                )
                                nc.scalar.copy(gt[:], gps_t[:, :HID])
                            else:
                                nc.gpsimd.indirect_dma_start(
                                    out=gt[:],
                                    out_offset=None,
                                    in_=y_full.ap(),
                                    in_offset=bass.IndirectOffsetOnAxis(
                                        ap=src_t[:, ck : ck + 1],
                                        axis=0,
                                    ),
                                    bounds_check=N_CORES * n_pad - 1,
                                    oob_is_err=False,
                                )
                            g_tiles = {ck: gt}
                        s_tile = sp.tile([P, P], mybir.dt.bfloat16, tag="s1h")
                        nc.vector.tensor_tensor(
                            out=s_tile[:],
                            in0=iota_t[:],
                            in1=dst_t[:, pi : pi + 1].to_broadcast([P, P]),
                            op=mybir.AluOpType.is_equal,
                        )
                        nc.tensor.matmul(
                            agg[:], g_tiles[ck][:], s_tile[:],
                            start=False, stop=(pi == p1 - 1),
                        )
                    nc.scalar.activation(
                        h1_t[:, w * P : (w + 1) * P], agg[:],
                        mybir.ActivationFunctionType.Relu,
                        bias=b1[:], scale=1.0,
                    )
                # pad-column constants: ypad = W1^T xpad ; h1pad = relu(+b1)
                ypps = mmp.tile([128, 1], f32, tag="mm128")
                nc.tensor.matmul(
                    ypps[:, 0:1], W1[:kdim, :], xpad[:kdim, 0:1],
                    start=True, stop=True,
                )
                h1pad = wp.tile([128, 1], f32, tag="h1pad")
                nc.scalar.activation(
                    h1pad[:], ypps[:, 0:1],
                    mybir.ActivationFunctionType.Relu, bias=b1[:], scale=1.0,
                )
                vpps = mmp.tile([128, 1], f32, tag="mm128")
                nc.tensor.matmul(
                    vpps[:, 0:1], W2[:], h1pad[:], start=True, stop=True
                )
                vpad = wp.tile([128, 1], f32, tag="vpad")
                nc.vector.tensor_copy(vpad[:], vpps[:, 0:1])
                # ---- W2 pass 1: partial sums / sumsq ----
                n_big = (n_pad + 511) // 512
                parts = wp.tile([128, 2 * n_big], f32, tag="parts")
                for b in range(n_big):
                    c0, c1 = b * 512, min((b + 1) * 512, n_pad)
                    wl = c1 - c0
                    h2ps = h2p.tile([128, 512], f32, tag="h2ps")
                    nc.tensor.matmul(
                        h2ps[:, :wl], W2[:], h1_t[:, c0:c1],
                        start=True, stop=True,
                    )
                    sq = wp.tile([128, 512], f32, tag="sq")
                    nc.scalar.activation(
                        sq[:, :wl], h2ps[:, :wl],
                        mybir.ActivationFunctionType.Square,
                    )
                    nc.vector.tensor_reduce(
                        parts[:, 2 * b : 2 * b + 1], h2ps[:, :wl],
                        mybir.AxisListType.X, mybir.AluOpType.add,
                    )
                    nc.vector.tensor_reduce(
                        parts[:, 2 * b + 1 : 2 * b + 2], sq[:, :wl],
                        mybir.AxisListType.X, mybir.AluOpType.add,
                    )
                stats = wp.tile([128, 2], f32, tag="stats")
                pv = parts[:].rearrange("p (b s) -> p s b", s=2)
                nc.vector.tensor_reduce(
                    stats[:, 0:1], pv[:, 0, :], mybir.AxisListType.X,
                    mybir.AluOpType.add,
                )
                nc.vector.tensor_reduce(
                    stats[:, 1:2], pv[:, 1, :], mybir.AxisListType.X,
                    mybir.AluOpType.add,
                )
                # subtract pad-column contribution: ndead * [v, v^2]
                ndead = wp.tile([128, 1], f32, tag="ndead")
                nc.vector.tensor_scalar(
                    out=ndead[:], in0=nrc_t[:], scalar1=-1.0,
                    scalar2=float(n_pad), op0=mybir.AluOpType.mult,
                    op1=mybir.AluOpType.add,
                )
                vsq = wp.tile([128, 1], f32, tag="vsq")
                nc.vector.tensor_tensor(
                    out=vsq[:], in0=vpad[:], in1=vpad[:],
                    op=mybir.AluOpType.mult,
                )
                corr = wp.tile([128, 2], f32, tag="corr")
                nc.vector.tensor_tensor(
                    out=corr[:, 0:1], in0=vpad[:], in1=ndead[:],
                    op=mybir.AluOpType.mult,
                )
                nc.vector.tensor_tensor(
                    out=corr[:, 1:2], in0=vsq[:], in1=ndead[:],
                    op=mybir.AluOpType.mult,
                )
                nc.vector.tensor_tensor(
                    out=stats[:], in0=stats[:], in1=corr[:],
                    op=mybir.AluOpType.subtract,
                )
                nc.sync.dma_start(stats_in[:, :], stats[:])
                nc.gpsimd.collective_compute(
                    "AllGather", mybir.AluOpType.bypass, replica_groups=rg,
                    ins=[stats_in.ap().opt()], outs=[stats_out.ap().opt()],
                )
                allst = wp.tile([128, N_CORES, 2], f32, tag="allst")
                nc.sync.dma_start(
                    allst[:], stats_out.ap().rearrange("(c p) s -> p c s", p=P)
                )
                tot = wp.tile([128, 2], f32, tag="tot")
                av = allst[:].rearrange("p c s -> p s c")
                nc.vector.tensor_reduce(
                    tot[:, 0:1], av[:, 0, :], mybir.AxisListType.X,
                    mybir.AluOpType.add,
                )
                nc.vector.tensor_reduce(
                    tot[:, 1:2], av[:, 1, :], mybir.AxisListType.X,
                    mybir.AluOpType.add,
                )
                inv_n = 1.0 / float(n_nodes)
                mu = wp.tile([128, 1], f32, tag="mu")
                nc.vector.tensor_scalar(
                    out=mu[:], in0=tot[:, 0:1], scalar1=inv_n, scalar2=None,
                    op0=mybir.AluOpType.mult,
                )
                var = wp.tile([128, 1], f32, tag="var")
                nc.vector.tensor_scalar(
                    out=var[:], in0=tot[:, 1:2], scalar1=inv_n, scalar2=None,
                    op0=mybir.AluOpType.mult,
                )
                musq = wp.tile([128, 1], f32, tag="musq")
                nc.vector.tensor_tensor(
                    out=musq[:], in0=mu[:], in1=mu[:], op=mybir.AluOpType.mult
                )
                nc.vector.tensor_tensor(
                    out=var[:], in0=var[:], in1=musq[:],
                    op=mybir.AluOpType.subtract,
                )
                nc.vector.tensor_scalar(
                    out=var[:], in0=var[:], scalar1=BN_EPS, scalar2=None,
                    op0=mybir.AluOpType.add,
                )
                sd = wp.tile([128, 1], f32, tag="sd")
                nc.scalar.sqrt(sd[:], var[:])
                inv_sd = wp.tile([128, 1], f32, tag="invsd")
                nc.vector.reciprocal(inv_sd[:], sd[:])
                A = wp.tile([128, 1], f32, tag="A")
                nc.vector.tensor_tensor(
                    out=A[:], in0=inv_sd[:], in1=wt[f"bn{li}g"][:],
                    op=mybir.AluOpType.mult,
                )
                negmuA = wp.tile([128, 1], f32, tag="negmuA")
                nc.vector.tensor_tensor(
                    out=negmuA[:], in0=mu[:], in1=A[:], op=mybir.AluOpType.mult
                )
                B = wp.tile([128, 1], f32, tag="B")
                nc.vector.tensor_tensor(
                    out=B[:], in0=wt[f"bn{li}b"][:], in1=negmuA[:],
                    op=mybir.AluOpType.subtract,
                )
                # ---- W2 pass 2 + BN + relu (in place: cur is dead) ----
                nxt = cur
                for b in range(n_big):
                    c0, c1 = b * 512, min((b + 1) * 512, n_pad)
                    wl = c1 - c0
                    h2ps = h2p.tile([128, 512], f32, tag="h2ps")
                    nc.tensor.matmul(
                        h2ps[:, :wl], W2[:], h1_t[:, c0:c1],
                        start=True, stop=True,
                    )
                    nc.scalar.activation(
                        nxt[:, c0:c1], h2ps[:, :wl],
                        mybir.ActivationFunctionType.Relu,
                        bias=B[:], scale=A[:],
                    )
                # update pad-column constant for next layer (in place)
                nc.scalar.activation(
                    xpad[:], vpad[:], mybir.ActivationFunctionType.Relu,
                    bias=B[:], scale=A[:],
                )
                cur = nxt

            # ================= pooling + lig MLP =================
            xg_ps = xgp.tile([128, gpc], f32, tag="xgps")
            for w in range(n_win):
                tps = mmp.tile([P, P], f32, tag="mm128")
                nc.tensor.transpose(tps[:], cur[:, w * P : (w + 1) * P], ident[:])
                x_nm = wp.tile([P, P], f32, tag="xnm")
                nc.vector.tensor_copy(x_nm[:], tps[:])
                pool1h = sp.tile([P, 256], f32, tag="pool1h")
                nc.vector.tensor_tensor(
                    out=pool1h[:],
                    in0=iota256_t[:],
                    in1=gcol_t[:, w : w + 1].to_broadcast([P, 256]),
                    op=mybir.AluOpType.is_equal,
                )
                nc.tensor.matmul(
                    xg_ps[:], x_nm[:], pool1h[:, :gpc],
                    start=(w == 0), stop=(w == n_win - 1),
                )
            xg_sb = pp.tile([128, gpc], f32)
            nc.vector.tensor_copy(xg_sb[:], xg_ps[:])
            lps = brp.tile([128, gpc], f32, tag="brps")
            nc.tensor.matmul(lps[:], wt["ligW"][:], xg_sb[:], start=True, stop=True)
            xg_t = pp.tile([128, gpc], f32)
            nc.scalar.activation(
                xg_t[:], lps[:], mybir.ActivationFunctionType.Relu,
                bias=wt["ligb"][:], scale=1.0,
            )

            # ================= head =================
            branches = [xg_t, xp_t, xa_t]
            xc1a = pp.tile([128, gpc], f32)
            xc1b = pp.tile([128, gpc], f32)
            for h, (xc1h, bkey) in enumerate(
                [(xc1a, "c1ba"), (xc1b, "c1bb")]
            ):
                hps = brp.tile([128, gpc], f32, tag="brps")
                for blk in range(3):
                    nc.tensor.matmul(
                        hps[:], wt[f"c1W{blk}{h}"][:],
                        branches[blk][:],
                        start=(blk == 0), stop=(blk == 2),
                    )
                nc.scalar.activation(
                    xc1h[:], hps[:], mybir.ActivationFunctionType.Relu,
                    bias=wt[bkey][:], scale=1.0,
                )
            c2ps = brp.tile([128, gpc], f32, tag="brps")
            nc.tensor.matmul(c2ps[:], wt["c2Wa"][:], xc1a[:], start=True, stop=False)
            nc.tensor.matmul(c2ps[:], wt["c2Wb"][:], xc1b[:], start=False, stop=True)
            xc2 = pp.tile([128, gpc], f32)
            nc.scalar.activation(
                xc2[:], c2ps[:], mybir.ActivationFunctionType.Relu,
                bias=wt["c2b"][:], scale=1.0,
            )
            ops = brp.tile([128, gpc], f32, tag="brps")
            nc.tensor.matmul(ops[:1, :], wt["outW"][:], xc2[:], start=True, stop=True)
            ovec = wp.tile([1, gpc], f32, tag="ovec")
            nc.scalar.activation(
                ovec[:], ops[:1, :], mybir.ActivationFunctionType.Identity,
                bias=wt["outb"][:], scale=1.0,
            )
            nc.sync.dma_start(out_d.ap(), ovec[:])

    return nc


def kernel(**inputs):
    meta, data = _build_host_data(
        inputs["x_ligand"], inputs["protein_seq"], inputs["a2h"],
        inputs["edge_index"], inputs["batch_ligand"],
    )
    gpc = meta["gpc"]
    key = (meta["n_pad"], meta["total_passes"], gpc, meta["seq_len"])
    if key not in _cache:
        nc = _build_program(meta)
        nc.compile()
        _cache[key] = nc
    nc = _cache[key]

    f32 = np.float32
    bf = np.dtype("bfloat16")
    iota128 = np.tile(np.arange(P, dtype=f32), (P, 1)).astype(bf)
    iota256 = np.tile(np.arange(256, dtype=f32), (P, 1))
    conv_k = np.asarray(inputs["conv_k"], f32)
    kblk = np.zeros((32, 128), f32)
    for g in range(4):
        kblk[g * 8 : (g + 1) * 8, g * 32 : (g + 1) * 32] = conv_k[:, 0, :].T
    convb_col = np.tile(np.asarray(inputs["conv_b"], f32), 4).reshape(128, 1)

    shared = {
        "iota128": iota128, "iota256": iota256, "kblk": kblk,
        "convb_col": convb_col,
    }
    col = lambda v: np.asarray(v, f32).reshape(-1, 1)
    for i in range(3):
        shared[f"g{i}W1"] = np.asarray(inputs[f"gin{i}_W1"], f32)
        shared[f"g{i}b1"] = col(inputs[f"gin{i}_b1"])
        shared[f"g{i}W2"] = np.asarray(inputs[f"gin{i}_W2"], f32)
        shared[f"g{i}b2"] = col(inputs[f"gin{i}_b2"])
        shared[f"bn{i}g"] = col(inputs[f"bn{i}_g"])
        shared[f"bn{i}b"] = col(inputs[f"bn{i}_b"])
    shared["ligW"] = np.asarray(inputs["lig_W"], f32)
    shared["ligb"] = col(inputs["lig_b"])
    shared["protW"] = np.asarray(inputs["prot_W"], f32)
    shared["protb"] = col(inputs["prot_b"])
    a2h1W = np.zeros((meta["a2h_pad"], HID), f32)
    a2h1W[: meta["a2h_dim"]] = np.asarray(inputs["a2h1_W"], f32)
    shared["a2h1W"] = a2h1W
    shared["a2h1b"] = col(inputs["a2h1_b"])
    shared["a2h2W"] = np.asarray(inputs["a2h2_W"], f32)
    shared["a2h2b"] = col(inputs["a2h2_b"])
    c1W = np.asarray(inputs["c1_W"], f32)
    for blk in range(3):
        for h in range(2):
            shared[f"c1W{blk}{h}"] = np.ascontiguousarray(
                c1W[blk * HID : (blk + 1) * HID, h * HID : (h + 1) * HID]
            )
    c1b = np.asarray(inputs["c1_b"], f32)
    shared["c1ba"] = col(c1b[:HID])
    shared["c1bb"] = col(c1b[HID:])
    c2W = np.asarray(inputs["c2_W"], f32)
    shared["c2Wa"] = np.ascontiguousarray(c2W[:HID, :])
    shared["c2Wb"] = np.ascontiguousarray(c2W[HID:, :])
    shared["c2b"] = col(inputs["c2_b"])
    shared["outW"] = np.asarray(inputs["out_W"], f32)
    shared["g0W1bf"] = np.asarray(inputs["gin0_W1"], f32).astype(np.dtype("bfloat16"))
    shared["outb"] = np.asarray(inputs["out_b"], f32).reshape(1, 1)

    in_maps = []
    for c in range(N_CORES):
        m = dict(shared)
        m["x_t"] = data["x_t"][c]
        m["src_tiles"] = data["src_tiles"][c]
        m["dst_tiles"] = data["dst_tiles"][c].astype(np.dtype("bfloat16"))
        m["graph_cols"] = data["graph_cols"][c]
        m["prot_c"] = data["prot_c"][c]
        m["a2h_t"] = data["a2h_t"][c]
        m["n_real_col"] = np.full((P, 1), float(meta["n_c"][c]), f32)
        m["x_gath"] = data["x_gath"][c].astype(np.dtype("bfloat16"))
        in_maps.append(m)

    res = run_bass_kernel_spmd(nc, in_maps, core_ids=list(range(N_CORES)))
    outs = [
        np.asarray(res.results[c]["out"]).reshape(gpc, 1)
        for c in range(N_CORES)
    ]
    return np.concatenate(outs, axis=0).astype(np.float32)


# revision 17
# speedup vs baseline: 1.2330x; 1.1436x over previous
"""Trainium2 8-core Bass kernel for A2HNet (GIN message passing + branches).

Self-contained: computes all sharding/index structures from the inputs at
call time, builds one SPMD Bass program, runs it on cores 0-7, and gathers
the full (2048, 1) output.

Sharding: graphs (and their nodes) are block-partitioned over the 8 cores.
Each GIN layer computes y = x @ W1 locally, AllGathers y (node-major) into a
replicated table, then each core aggregates its own nodes' incoming edges
with an indirect-DMA row gather feeding one-hot matmuls that accumulate
y[dst] + sum_e y[src_e] directly in PSUM. BatchNorm batch stats are
AllGathered as per-core partial sums. The protein-conv / a2h / head branches
are computed per-core on the local 256 graphs.
"""
import numpy as np

from concourse import bacc, bass, mybir, tile
from concourse.bass_utils import run_bass_kernel_spmd
from concourse.masks import make_identity

N_CORES = 8
HID = 128
BN_EPS = 1e-5
P = 128
GCHUNK = 1  # passes per indirect gather call (128 rows)

_cache = {}


def _build_host_data(x_ligand, protein_seq, a2h, edge_index, batch_ligand):
    n_nodes = x_ligand.shape[0]
    n_graphs = a2h.shape[0]
    gpc = n_graphs // N_CORES

    batch = np.asarray(batch_ligand).astype(np.int64)
    src_all = np.asarray(edge_index)[0].astype(np.int64)
    dst_all = np.asarray(edge_index)[1].astype(np.int64)

    node_core = batch // gpc
    core_starts = np.searchsorted(node_core, np.arange(N_CORES))
    core_ends = np.searchsorted(node_core, np.arange(N_CORES), side="right")
    n_c = core_ends - core_starts
    n_pad = int(np.ceil(n_c.max() / P) * P)
    n_win = n_pad // P

    # relabel nodes per core by descending in-degree: equalizes per-window
    # edge counts across cores, shrinking max-over-cores pass padding
    deg = np.bincount(dst_all, minlength=n_nodes)
    local_old = np.arange(n_nodes) - core_starts[node_core]
    local_idx = np.empty(n_nodes, np.int64)
    perms = []
    for c in range(N_CORES):
        s_, e_ = int(core_starts[c]), int(core_ends[c])
        perm = np.argsort(-deg[s_:e_], kind="stable")
        invp = np.argsort(perm, kind="stable")
        local_idx[s_:e_] = invp
        perms.append(perm)
    gid = node_core * n_pad + local_idx

    edge_core = node_core[dst_all]
    counts = np.zeros((N_CORES, n_win), np.int64)
    per_core_edges = []
    for c in range(N_CORES):
        m = edge_core == c
        e_src = src_all[m]
        e_dst_loc = local_idx[dst_all[m]]
        order = np.argsort(e_dst_loc, kind="stable")
        e_src, e_dst_loc = e_src[order], e_dst_loc[order]
        counts[c] = np.bincount(e_dst_loc // P, minlength=n_win)
        per_core_edges.append((e_src, e_dst_loc))

    passes_w = np.maximum(1, np.ceil(counts.max(axis=0) / P).astype(np.int64))
    total_passes = int(passes_w.sum())
    rem = (-total_passes) % GCHUNK
    passes_w[-1] += rem
    total_passes += rem
    win_pass_start = np.zeros(n_win + 1, np.int64)
    win_pass_start[1:] = np.cumsum(passes_w)

    src_tiles = np.full((N_CORES, P, total_passes), 1 << 22, np.int32)
    dst_tiles = np.full((N_CORES, P, total_passes), 300.0, np.float32)
    for c in range(N_CORES):
        e_src, e_dst_loc = per_core_edges[c]
        w_of = e_dst_loc // P
        off_in_w = np.arange(len(e_src)) - np.searchsorted(w_of, w_of)
        slot = win_pass_start[w_of] * P + off_in_w
        src_tiles[c, slot % P, slot // P] = gid[e_src]
        dst_tiles[c, slot % P, slot // P] = (e_dst_loc % P).astype(np.float32)

    x_t = np.zeros((N_CORES, 128, n_pad), np.float32)
    graph_cols = np.full((N_CORES, P, n_win), 300.0, np.float32)
    xl = np.asarray(x_ligand).astype(np.float32)
    for c in range(N_CORES):
        s, e = int(core_starts[c]), int(core_ends[c])
        x_t[c, :78, : e - s] = xl[s + perms[c]].T
        col = np.full(n_pad, 300.0, np.float32)
        col[: e - s] = (batch[s + perms[c]] - c * gpc).astype(np.float32)
        graph_cols[c] = col.reshape(n_win, P).T

    prot = np.asarray(protein_seq).astype(np.float32).reshape(n_graphs, -1)
    seq_len = prot.shape[1]
    a2h_flat = np.asarray(a2h).astype(np.float32).reshape(n_graphs, -1)
    a2h_dim = a2h_flat.shape[1]
    a2h_pad = int(np.ceil(a2h_dim / P) * P)
    prot_c = np.ascontiguousarray(prot.reshape(N_CORES, gpc, seq_len))
    a2h_t = np.zeros((N_CORES, a2h_pad, gpc), np.float32)
    for c in range(N_CORES):
        a2h_t[c, :a2h_dim, :] = a2h_flat[c * gpc : (c + 1) * gpc].T

    x_gath = np.zeros((N_CORES, 78, total_passes * P), np.float32)
    inv = np.full(N_CORES * n_pad, -1, np.int64)
    for c in range(N_CORES):
        s_, e_ = int(core_starts[c]), int(core_ends[c])
        inv[c * n_pad : c * n_pad + (e_ - s_)] = s_ + perms[c]
    for c in range(N_CORES):
        st = src_tiles[c]  # (P, total_passes), gid or 1<<22
        flat = st.T.reshape(-1)  # slot (pi, p) at pi*P + p
        valid = flat < N_CORES * n_pad
        rows = inv[flat[valid]]
        cols = np.where(valid)[0]
        x_gath[c][:, cols] = xl[rows].T
    meta = dict(
        n_pad=n_pad, n_win=n_win, gpc=gpc, n_nodes=n_nodes,
        total_passes=total_passes, win_pass_start=win_pass_start,
        seq_len=seq_len, a2h_pad=a2h_pad, n_c=n_c, a2h_dim=a2h_dim,
    )
    data = dict(
        x_t=x_t, src_tiles=src_tiles, dst_tiles=dst_tiles,
        graph_cols=graph_cols, prot_c=prot_c, a2h_t=a2h_t, x_gath=x_gath,
    )
    return meta, data


def _build_program(meta):
    n_pad, n_win, gpc = meta["n_pad"], meta["n_win"], meta["gpc"]
    total_passes = meta["total_passes"]
    wps = meta["win_pass_start"]
    seq_len, a2h_pad = meta["seq_len"], meta["a2h_pad"]
    n_nodes = meta["n_nodes"]
    conv_w = seq_len - 8 + 1
    g_grp = gpc // 4
    f32 = mybir.dt.float32

    nc = bacc.Bacc(None, target_bir_lowering=False, num_devices=N_CORES)

    def par(name, shape, dt=f32):
        return nc.declare_dram_parameter(name, list(shape), dt, isOutput=False)

    x_t_d = par("x_t", (128, n_pad))
    src_d = par("src_tiles", (P, total_passes), mybir.dt.int32)
    dst_d = par("dst_tiles", (P, total_passes), mybir.dt.bfloat16)
    gcol_d = par("graph_cols", (P, n_win))
    prot_d = par("prot_c", (gpc, seq_len))
    a2h_d = par("a2h_t", (a2h_pad, gpc))
    iota_d = par("iota128", (P, P), mybir.dt.bfloat16)
    iota256_d = par("iota256", (P, 256))
    kblk_d = par("kblk", (32, 128))
    convb_d = par("convb_col", (128, 1))
    nreal_d = par("n_real_col", (P, 1))
    xgath_d = par("x_gath", (78, total_passes * P), mybir.dt.bfloat16)
    g0w1bf_d = par("g0W1bf", (78, HID), mybir.dt.bfloat16)
    w_d = {}
    for i in range(3):
        din = 78 if i == 0 else HID
        for nm, shp in [
            (f"g{i}W1", (din, HID)), (f"g{i}b1", (HID, 1)),
            (f"g{i}W2", (HID, HID)), (f"g{i}b2", (HID, 1)),
            (f"bn{i}g", (HID, 1)), (f"bn{i}b", (HID, 1)),
        ]:
            w_d[nm] = par(nm, shp)
    for nm, shp in [
        ("ligW", (HID, HID)), ("ligb", (HID, 1)),
        ("protW", (32, HID)), ("protb", (HID, 1)),
        ("a2h1W", (a2h_pad, HID)), ("a2h1b", (HID, 1)),
        ("a2h2W", (HID, HID)), ("a2h2b", (HID, 1)),
        ("c1W00", (HID, HID)), ("c1W10", (HID, HID)), ("c1W20", (HID, HID)),
        ("c1W01", (HID, HID)), ("c1W11", (HID, HID)), ("c1W21", (HID, HID)),
        ("c1ba", (HID, 1)), ("c1bb", (HID, 1)),
        ("c2Wa", (HID, HID)), ("c2Wb", (HID, HID)), ("c2b", (HID, 1)),
        ("outW", (HID, 1)), ("outb", (1, 1)),
    ]:
        w_d[nm] = par(nm, shp)
    out_d = nc.declare_dram_parameter("out", [gpc, 1], f32, isOutput=True)

    bf16 = mybir.dt.bfloat16
    y_nm = nc.dram_tensor("y_nm", [n_pad, HID], bf16)
    y_full = nc.dram_tensor(
        "y_full", [N_CORES * n_pad, HID], bf16, addr_space="Shared"
    )
    stats_in = nc.dram_tensor("stats_in", [P, 2], f32)
    stats_out = nc.dram_tensor(
        "stats_out", [N_CORES * P, 2], f32, addr_space="Shared"
    )
    rg = [list(range(N_CORES))]

    with tile.TileContext(nc) as tc:
        with (
            tc.tile_pool(name="persist", bufs=1) as pp,
            tc.tile_pool(name="work", bufs=3) as wp,
            tc.tile_pool(name="patchp", bufs=1) as patchp,
            tc.tile_pool(name="gpool", bufs=16) as gp,
            tc.tile_pool(name="spool", bufs=16) as sp,
            tc.tile_pool(name="ypool", bufs=6) as yp,
            tc.tile_pool(name="mm128", bufs=3, space="PSUM") as mmp,
            tc.tile_pool(name="h2p", bufs=2, space="PSUM") as h2p,
            tc.tile_pool(name="convp", bufs=1, space="PSUM") as convp,
            tc.tile_pool(name="brp", bufs=1, space="PSUM") as brp,
            tc.tile_pool(name="xgp", bufs=1, space="PSUM") as xgp,
        ):
            # ---- persistent loads ----
            cur = pp.tile([128, n_pad], f32, tag="cur")
            nc.sync.dma_start(cur[:], x_t_d.ap())
            h1_t = pp.tile([128, n_pad], f32)
            src_t = pp.tile([P, total_passes], mybir.dt.int32)
            nc.sync.dma_start(src_t[:], src_d.ap())
            dst_t = pp.tile([P, total_passes], mybir.dt.bfloat16)
            nc.sync.dma_start(dst_t[:], dst_d.ap())
            gcol_t = pp.tile([P, n_win], f32)
            nc.sync.dma_start(gcol_t[:], gcol_d.ap())
            iota_t = pp.tile([P, P], mybir.dt.bfloat16)
            nc.sync.dma_start(iota_t[:], iota_d.ap())
            iota256_t = pp.tile([P, 256], f32)
            nc.sync.dma_start(iota256_t[:], iota256_d.ap())
            kblk_t = pp.tile([32, 128], f32)
            nc.sync.dma_start(kblk_t[:], kblk_d.ap())
            convb_t = pp.tile([128, 1], f32)
            nc.sync.dma_start(convb_t[:], convb_d.ap())
            nrc_t = pp.tile([P, 1], f32)
            nc.sync.dma_start(nrc_t[:], nreal_d.ap())
            for _gi in range(16):
                gz = gp.tile([P, HID], mybir.dt.bfloat16, tag="gt", name=f"gz{_gi}")
                nc.vector.memset(gz[:], 0.0)
            wt = {}
            for k, d in w_d.items():
                if k == "a2h1W":
                    continue
                wt[k] = pp.tile(list(d.shape), f32, tag=f"w_{k}", name=f"w_{k}")
                nc.sync.dma_start(wt[k][:], d.ap())
            g0w1bf = pp.tile([78, HID], mybir.dt.bfloat16)
            nc.sync.dma_start(g0w1bf[:], g0w1bf_d.ap())
            ident = pp.tile([P, P], f32)
            make_identity(nc, ident[:])

            # ================= protein conv branch =================
            maxes = pp.tile([128, g_grp], f32)
            for grp in range(g_grp):
                patch = patchp.tile([32, conv_w], f32, tag="patch")
                sl = prot_d.ap()[4 * grp : 4 * grp + 4, :]
                src_ap = bass.AP(
                    sl.tensor, sl.offset, [list(sl.ap[0]), [1, 8], [1, conv_w]]
                )
                nc.sync.dma_start(patch[:], src_ap)
                m2 = wp.tile([128, 2], f32, tag="m2")
                for half, (c0, c1) in enumerate([(0, 512), (512, conv_w)]):
                    cps = convp.tile([128, 512], f32, tag="convps")
                    nc.tensor.matmul(
                        cps[:, : c1 - c0], kblk_t[:], patch[:, c0:c1],
                        start=True, stop=True,
                    )
                    nc.vector.tensor_reduce(
                        m2[:, half : half + 1], cps[:, : c1 - c0],
                        mybir.AxisListType.X, mybir.AluOpType.max,
                    )
                nc.vector.tensor_reduce(
                    maxes[:, grp : grp + 1], m2[:], mybir.AxisListType.X,
                    mybir.AluOpType.max,
                )
            xp_m = pp.tile([128, g_grp], f32)
            nc.scalar.activation(
                xp_m[:], maxes[:], mybir.ActivationFunctionType.Relu,
                bias=convb_t[:], scale=1.0,
            )
            xp_ch = pp.tile([32, gpc], f32)
            xp_ch_v = xp_ch[:].rearrange("c (g gi) -> c g gi", gi=4)
            for gi in range(4):
                nc.sync.dma_start(
                    xp_ch_v[:, :, gi], xp_m[gi * 32 : (gi + 1) * 32, :]
                )
            xp_t = pp.tile([128, gpc], f32)
            pps = brp.tile([128, gpc], f32, tag="brps")
            nc.tensor.matmul(pps[:], wt["protW"][:], xp_ch[:], start=True, stop=True)
            nc.scalar.activation(
                xp_t[:], pps[:], mybir.ActivationFunctionType.Relu,
                bias=wt["protb"][:], scale=1.0,
            )

            # ================= a2h branch =================
            xa1 = pp.tile([128, gpc], f32)
            aps = brp.tile([128, gpc], f32, tag="brps")
            n_chunk = a2h_pad // P
            for ch in range(n_chunk):
                a_tile = wp.tile([P, gpc], f32, tag="a2h")
                nc.sync.dma_start(a_tile[:], a2h_d.ap()[ch * P : (ch + 1) * P, :])
                aw = wp.tile([P, HID], f32, tag="a2hw")
                nc.sync.dma_start(
                    aw[:], w_d["a2h1W"].ap()[ch * P : (ch + 1) * P, :]
                )
                nc.tensor.matmul(
                    aps[:], aw[:], a_tile[:],
                    start=(ch == 0), stop=(ch == n_chunk - 1),
                )
            nc.scalar.activation(
                xa1[:], aps[:], mybir.ActivationFunctionType.Relu,
                bias=wt["a2h1b"][:], scale=1.0,
            )
            xa_t = pp.tile([128, gpc], f32)
            aps2 = brp.tile([128, gpc], f32, tag="brps")
            nc.tensor.matmul(aps2[:], wt["a2h2W"][:], xa1[:], start=True, stop=True)
            nc.scalar.activation(
                xa_t[:], aps2[:], mybir.ActivationFunctionType.Relu,
                bias=wt["a2h2b"][:], scale=1.0,
            )

            # ================= GIN layers =================
            # xpad_col: value of pad-node activation columns (constant per
            # feature). Starts at zero (host zero-pads x).
            xpad = pp.tile([128, 1], f32)
            nc.vector.memset(xpad[:], 0.0)
            for li in range(3):
                kdim = 78 if li == 0 else HID
                W1, W2 = wt[f"g{li}W1"], wt[f"g{li}W2"]
                b1, b2 = wt[f"g{li}b1"], wt[f"g{li}b2"]
                # ---- y node-major -> DRAM, then AllGather (layers 1,2) ----
                if li > 0:
                    for w in range(n_win):
                        yps = mmp.tile([P, HID], f32, tag="mm128")
                        nc.tensor.matmul(
                            yps[:], cur[:kdim, w * P : (w + 1) * P],
                            W1[:kdim, :], start=True, stop=True,
                        )
                        y_sb = yp.tile([P, HID], mybir.dt.bfloat16, tag="ysb")
                        if w % 2 == 0:
                            nc.vector.tensor_copy(y_sb[:], yps[:])
                        else:
                            nc.scalar.copy(y_sb[:], yps[:])
                        nc.sync.dma_start(y_nm[w * P : (w + 1) * P, :], y_sb[:])
                    nc.gpsimd.collective_compute(
                        "AllGather", mybir.AluOpType.bypass, replica_groups=rg,
                        ins=[y_nm.ap().opt()], outs=[y_full.ap().opt()],
                    )
                # ---- aggregation ----
                g_tiles = {}
                xg_blks = {}
                for w in range(n_win):
                    agg = mmp.tile([128, P], f32, tag="mm128")
                    nc.tensor.matmul(
                        agg[:], W1[:kdim, :], cur[:kdim, w * P : (w + 1) * P],
                        start=True, stop=False,
                    )
                    p0, p1 = int(wps[w]), int(wps[w + 1])
                    for pi in range(p0, p1):
                        ck = pi // GCHUNK
                        if ck not in g_tiles:
                            gt = gp.tile([P, HID], mybir.dt.bfloat16, tag="gt")
                            if li == 0:
                                blk8 = ck // 8
                                if blk8 not in xg_blks:
                                    xg_sl = wp.tile(
                                        [78, 8 * P], mybir.dt.bfloat16,
                                        tag="xgsl"
                                    )
                                    lo = blk8 * 8 * P
                                    hi = min(lo + 8 * P, total_passes * P)
                                    nc.sync.dma_start(
                                        xg_sl[:, : hi - lo],
                                        xgath_d.ap()[:, lo:hi],
                                    )
                                    xg_blks = {blk8: xg_sl}
                                xs = xg_blks[blk8]
                                o = (ck % 8) * P
                                gps_t = h2p.tile([128, 512], f32, tag="h2ps")
                                nc.tensor.matmul(
                                    gps_t[:, :HID], xs[:, o : o + P],
                                    g0w1bf[:], start=True, stop=True,
                                )
                                nc.scalar.copy(gt[:], gps_t[:, :HID])
                            else:
                                nc.gpsimd.indirect_dma_start(
                                    out=gt[:],
                                    out_offset=None,
                                    in_=y_full.ap(),
                                    in_offset=bass.IndirectOffsetOnAxis(
                                        ap=src_t[:, ck : ck + 1],
                                        axis=0,
                                    ),
                                    bounds_check=N_CORES * n_pad - 1,
                                    oob_is_err=False,
                                )
                            g_tiles = {ck: gt}
                        s_tile = sp.tile([P, P], mybir.dt.bfloat16, tag="s1h")
                        nc.vector.tensor_tensor(
                            out=s_tile[:],
                            in0=iota_t[:],
                            in1=dst_t[:, pi : pi + 1].to_broadcast([P, P]),
                            op=mybir.AluOpType.is_equal,
                        )
                        nc.tensor.matmul(
                            agg[:], g_tiles[ck][:], s_tile[:],
                            start=False, stop=(pi == p1 - 1),
                        )
                    nc.scalar.activation(
                        h1_t[:, w * P : (w + 1) * P], agg[:],
                        mybir.ActivationFunctionType.Relu,
                        bias=b1[:], scale=1.0,
                    )
                # pad-column constants: ypad = W1^T xpad ; h1pad = relu(+b1)
                ypps = mmp.tile([128, 1], f32, tag="mm128")
                nc.tensor.matmul(
                    ypps[:, 0:1], W1[:kdim, :], xpad[:kdim, 0:1],
                    start=True, stop=True,
                )
                h1pad = wp.tile([128, 1], f32, tag="h1pad")
                nc.scalar.activation(
                    h1pad[:], ypps[:, 0:1],
                    mybir.ActivationFunctionType.Relu, bias=b1[:], scale=1.0,
                )
                vpps = mmp.tile([128, 1], f32, tag="mm128")
                nc.tensor.matmul(
                    vpps[:, 0:1], W2[:], h1pad[:], start=True, stop=True
                )
                vpad = wp.tile([128, 1], f32, tag="vpad")
                nc.vector.tensor_copy(vpad[:], vpps[:, 0:1])
                # ---- W2 pass 1: partial sums / sumsq ----
                n_big = (n_pad + 511) // 512
                parts = wp.tile([128, 2 * n_big], f32, tag="parts")
                for b in range(n_big):
                    c0, c1 = b * 512, min((b + 1) * 512, n_pad)
                    wl = c1 - c0
                    h2ps = h2p.tile([128, 512], f32, tag="h2ps")
                    nc.tensor.matmul(
                        h2ps[:, :wl], W2[:], h1_t[:, c0:c1],
                        start=True, stop=True,
                    )
                    sq = wp.tile([128, 512], f32, tag="sq")
                    nc.scalar.activation(
                        sq[:, :wl], h2ps[:, :wl],
                        mybir.ActivationFunctionType.Square,
                    )
                    nc.vector.tensor_reduce(
                        parts[:, 2 * b : 2 * b + 1], h2ps[:, :wl],
                        mybir.AxisListType.X, mybir.AluOpType.add,
                    )
                    nc.vector.tensor_reduce(
                        parts[:, 2 * b + 1 : 2 * b + 2], sq[:, :wl],
                        mybir.AxisListType.X, mybir.AluOpType.add,
                    )
                stats = wp.tile([128, 2], f32, tag="stats")
                pv = parts[:].rearrange("p (b s) -> p s b", s=2)
                nc.vector.tensor_reduce(
                    stats[:, 0:1], pv[:, 0, :], mybir.AxisListType.X,
                    mybir.AluOpType.add,
                )
                nc.vector.tensor_reduce(
                    stats[:, 1:2], pv[:, 1, :], mybir.AxisListType.X,
                    mybir.AluOpType.add,
                )
                # subtract pad-column contribution: ndead * [v, v^2]
                ndead = wp.tile([128, 1], f32, tag="ndead")
                nc.vector.tensor_scalar(
                    out=ndead[:], in0=nrc_t[:], scalar1=-1.0,
                    scalar2=float(n_pad), op0=mybir.AluOpType.mult,
                    op1=mybir.AluOpType.add,
                )
                vsq = wp.tile([128, 1], f32, tag="vsq")
                nc.vector.tensor_tensor(
                    out=vsq[:], in0=vpad[:], in1=vpad[:],
                    op=mybir.AluOpType.mult,
                )
                corr = wp.tile([128, 2], f32, tag="corr")
                nc.vector.tensor_tensor(
                    out=corr[:, 0:1], in0=vpad[:], in1=ndead[:],
                    op=mybir.AluOpType.mult,
                )
                nc.vector.tensor_tensor(
                    out=corr[:, 1:2], in0=vsq[:], in1=ndead[:],
                    op=mybir.AluOpType.mult,
                )
                nc.vector.tensor_tensor(
                    out=stats[:], in0=stats[:], in1=corr[:],
                    op=mybir.AluOpType.subtract,
                )
                nc.sync.dma_start(stats_in[:, :], stats[:])
                nc.gpsimd.collective_compute(
                    "AllGather", mybir.AluOpType.bypass, replica_groups=rg,
                    ins=[stats_in.ap().opt()], outs=[stats_out.ap().opt()],
                )
                allst = wp.tile([128, N_CORES, 2], f32, tag="allst")
                nc.sync.dma_start(
                    allst[:], stats_out.ap().rearrange("(c p) s -> p c s", p=P)
                )
                tot = wp.tile([128, 2], f32, tag="tot")
                av = allst[:].rearrange("p c s -> p s c")
                nc.vector.tensor_reduce(
                    tot[:, 0:1], av[:, 0, :], mybir.AxisListType.X,
                    mybir.AluOpType.add,
                )
                nc.vector.tensor_reduce(
                    tot[:, 1:2], av[:, 1, :], mybir.AxisListType.X,
                    mybir.AluOpType.add,
                )
                inv_n = 1.0 / float(n_nodes)
                mu = wp.tile([128, 1], f32, tag="mu")
                nc.vector.tensor_scalar(
                    out=mu[:], in0=tot[:, 0:1], scalar1=inv_n, scalar2=None,
                    op0=mybir.AluOpType.mult,
                )
                var = wp.tile([128, 1], f32, tag="var")
                nc.vector.tensor_scalar(
                    out=var[:], in0=tot[:, 1:2], scalar1=inv_n, scalar2=None,
                    op0=mybir.AluOpType.mult,
                )
                musq = wp.tile([128, 1], f32, tag="musq")
                nc.vector.tensor_tensor(
                    out=musq[:], in0=mu[:], in1=mu[:], op=mybir.AluOpType.mult
                )
                nc.vector.tensor_tensor(
                    out=var[:], in0=var[:], in1=musq[:],
                    op=mybir.AluOpType.subtract,
                )
                nc.vector.tensor_scalar(
                    out=var[:], in0=var[:], scalar1=BN_EPS, scalar2=None,
                    op0=mybir.AluOpType.add,
                )
                sd = wp.tile([128, 1], f32, tag="sd")
                nc.scalar.sqrt(sd[:], var[:])
                inv_sd = wp.tile([128, 1], f32, tag="invsd")
                nc.vector.reciprocal(inv_sd[:], sd[:])
                A = wp.tile([128, 1], f32, tag="A")
                nc.vector.tensor_tensor(
                    out=A[:], in0=inv_sd[:], in1=wt[f"bn{li}g"][:],
                    op=mybir.AluOpType.mult,
                )
                negmuA = wp.tile([128, 1], f32, tag="negmuA")
                nc.vector.tensor_tensor(
                    out=negmuA[:], in0=mu[:], in1=A[:], op=mybir.AluOpType.mult
                )
                B = wp.tile([128, 1], f32, tag="B")
                nc.vector.tensor_tensor(
                    out=B[:], in0=wt[f"bn{li}b"][:], in1=negmuA[:],
                    op=mybir.AluOpType.subtract,
                )
                # ---- W2 pass 2 + BN + relu (in place: cur is dead) ----
                nxt = cur
                for b in range(n_big):
                    c0, c1 = b * 512, min((b + 1) * 512, n_pad)
                    wl = c1 - c0
                    h2ps = h2p.tile([128, 512], f32, tag="h2ps")
                    nc.tensor.matmul(
                        h2ps[:, :wl], W2[:], h1_t[:, c0:c1],
                        start=True, stop=True,
                    )
                    nc.scalar.activation(
                        nxt[:, c0:c1], h2ps[:, :wl],
                        mybir.ActivationFunctionType.Relu,
                        bias=B[:], scale=A[:],
                    )
                # update pad-column constant for next layer (in place)
                nc.scalar.activation(
                    xpad[:], vpad[:], mybir.ActivationFunctionType.Relu,
                    bias=B[:], scale=A[:],
                )
                cur = nxt

            # ================= pooling + lig MLP =================
            xg_ps = xgp.tile([128, gpc], f32, tag="xgps")
            for w in range(n_win):
                tps = mmp.tile([P, P], f32, tag="mm128")
                nc.tensor.transpose(tps[:], cur[:, w * P : (w + 1) * P], ident[:])
                x_nm = wp.tile([P, P], f32, tag="xnm")
                nc.vector.tensor_copy(x_nm[:], tps[:])
                pool1h = sp.tile([P, 256], f32, tag="pool1h")
                nc.vector.tensor_tensor(
                    out=pool1h[:],
                    in0=iota256_t[:],
                    in1=gcol_t[:, w : w + 1].to_broadcast([P, 256]),
                    op=mybir.AluOpType.is_equal,
                )
                nc.tensor.matmul(
                    xg_ps[:], x_nm[:], pool1h[:, :gpc],
                    start=(w == 0), stop=(w == n_win - 1),
                )
            xg_sb = pp.tile([128, gpc], f32)
            nc.vector.tensor_copy(xg_sb[:], xg_ps[:])
            lps = brp.tile([128, gpc], f32, tag="brps")
            nc.tensor.matmul(lps[:], wt["ligW"][:], xg_sb[:], start=True, stop=True)
            xg_t = pp.tile([128, gpc], f32)
            nc.scalar.activation(
                xg_t[:], lps[:], mybir.ActivationFunctionType.Relu,
                bias=wt["ligb"][:], scale=1.0,
            )

            # ================= head =================
            branches = [xg_t, xp_t, xa_t]
            xc1a = pp.tile([128, gpc], f32)
            xc1b = pp.tile([128, gpc], f32)
            for h, (xc1h, bkey) in enumerate(
                [(xc1a, "c1ba"), (xc1b, "c1bb")]
            ):
                hps = brp.tile([128, gpc], f32, tag="brps")
                for blk in range(3):
                    nc.tensor.matmul(
                        hps[:], wt[f"c1W{blk}{h}"][:],
                        branches[blk][:],
                        start=(blk == 0), stop=(blk == 2),
                    )
                nc.scalar.activation(
                    xc1h[:], hps[:], mybir.ActivationFunctionType.Relu,
                    bias=wt[bkey][:], scale=1.0,
                )
            c2ps = brp.tile([128, gpc], f32, tag="brps")
            nc.tensor.matmul(c2ps[:], wt["c2Wa"][:], xc1a[:], start=True, stop=False)
            nc.tensor.matmul(c2ps[:], wt["c2Wb"][:], xc1b[:], start=False, stop=True)
            xc2 = pp.tile([128, gpc], f32)
            nc.scalar.activation(
                xc2[:], c2ps[:], mybir.ActivationFunctionType.Relu,
                bias=wt["c2b"][:], scale=1.0,
            )
            ops = brp.tile([128, gpc], f32, tag="brps")
            nc.tensor.matmul(ops[:1, :], wt["outW"][:], xc2[:], start=True, stop=True)
            ovec = wp.tile([1, gpc], f32, tag="ovec")
            nc.scalar.activation(
                ovec[:], ops[:1, :], mybir.ActivationFunctionType.Identity,
                bias=wt["outb"][:], scale=1.0,
            )
            nc.sync.dma_start(out_d.ap(), ovec[:])

    return nc


def kernel(**inputs):
    meta, data = _build_host_data(
        inputs["x_ligand"], inputs["protein_seq"], inputs["a2h"],
        inputs["edge_index"], inputs["batch_ligand"],
    )
    gpc = meta["gpc"]
    key = (meta["n_pad"], meta["total_passes"], gpc, meta["seq_len"])
    if key not in _cache:
        nc = _build_program(meta)
        nc.compile()
        _cache[key] = nc
    nc = _cache[key]

    f32 = np.float32
    bf = np.dtype("bfloat16")
    iota128 = np.tile(np.arange(P, dtype=f32), (P, 1)).astype(bf)
    iota256 = np.tile(np.arange(256, dtype=f32), (P, 1))
    conv_k = np.asarray(inputs["conv_k"], f32)
    kblk = np.zeros((32, 128), f32)
    for g in range(4):
        kblk[g * 8 : (g + 1) * 8, g * 32 : (g + 1) * 32] = conv_k[:, 0, :].T
    convb_col = np.tile(np.asarray(inputs["conv_b"], f32), 4).reshape(128, 1)

    shared = {
        "iota128": iota128, "iota256": iota256, "kblk": kblk,
        "convb_col": convb_col,
    }
    col = lambda v: np.asarray(v, f32).reshape(-1, 1)
    for i in range(3):
        shared[f"g{i}W1"] = np.asarray(inputs[f"gin{i}_W1"], f32)
        shared[f"g{i}b1"] = col(inputs[f"gin{i}_b1"])
        shared[f"g{i}W2"] = np.asarray(inputs[f"gin{i}_W2"], f32)
        shared[f"g{i}b2"] = col(inputs[f"gin{i}_b2"])
        shared[f"bn{i}g"] = col(inputs[f"bn{i}_g"])
        shared[f"bn{i}b"] = col(inputs[f"bn{i}_b"])
    shared["ligW"] = np.asarray(inputs["lig_W"], f32)
    shared["ligb"] = col(inputs["lig_b"])
    shared["protW"] = np.asarray(inputs["prot_W"], f32)
    shared["protb"] = col(inputs["prot_b"])
    a2h1W = np.zeros((meta["a2h_pad"], HID), f32)
    a2h1W[: meta["a2h_dim"]] = np.asarray(inputs["a2h1_W"], f32)
    shared["a2h1W"] = a2h1W
    shared["a2h1b"] = col(inputs["a2h1_b"])
    shared["a2h2W"] = np.asarray(inputs["a2h2_W"], f32)
    shared["a2h2b"] = col(inputs["a2h2_b"])
    c1W = np.asarray(inputs["c1_W"], f32)
    for blk in range(3):
        for h in range(2):
            shared[f"c1W{blk}{h}"] = np.ascontiguousarray(
                c1W[blk * HID : (blk + 1) * HID, h * HID : (h + 1) * HID]
            )
    c1b = np.asarray(inputs["c1_b"], f32)
    shared["c1ba"] = col(c1b[:HID])
    shared["c1bb"] = col(c1b[HID:])
    c2W = np.asarray(inputs["c2_W"], f32)
    shared["c2Wa"] = np.ascontiguousarray(c2W[:HID, :])
    shared["c2Wb"] = np.ascontiguousarray(c2W[HID:, :])
    shared["c2b"] = col(inputs["c2_b"])
    shared["outW"] = np.asarray(inputs["out_W"], f32)
    shared["g0W1bf"] = np.asarray(inputs["gin0_W1"], f32).astype(np.dtype("bfloat16"))
    shared["outb"] = np.asarray(inputs["out_b"], f32).reshape(1, 1)

    in_maps = []
    for c in range(N_CORES):
        m = dict(shared)
        m["x_t"] = data["x_t"][c]
        m["src_tiles"] = data["src_tiles"][c]
        m["dst_tiles"] = data["dst_tiles"][c].astype(np.dtype("bfloat16"))
        m["graph_cols"] = data["graph_cols"][c]
        m["prot_c"] = data["prot_c"][c]
        m["a2h_t"] = data["a2h_t"][c]
        m["n_real_col"] = np.full((P, 1), float(meta["n_c"][c]), f32)
        m["x_gath"] = data["x_gath"][c].astype(np.dtype("bfloat16"))
        in_maps.append(m)

    res = run_bass_kernel_spmd(nc, in_maps, core_ids=list(range(N_CORES)))
    outs = [
        np.asarray(res.results[c]["out"]).reshape(gpc, 1)
        for c in range(N_CORES)
    ]
    return np.concatenate(outs, axis=0).astype(np.float32)
